# revision 1
# baseline (speedup 1.0000x reference)
"""LoRA-MoE Linear kernel for Trainium2, 8-core SPMD.

Strategy (token-parallel):
  - Shard the 8192 tokens across 8 cores (1024 each). Host pre-transposes
    x shards and the dense weight so every matmul operand arrives with the
    contraction dim on SBUF partitions (no on-chip transposes of big data).
  - Per core: gate MLP -> top-2 routing -> local expert counts -> tiny
    AllReduce (global capacity rule) -> combine weights; LoRA down-proj
    (tmp^T = A_cat^T x) computed in [er, tok] layout; main matmul runs
    W-stationary / x-moving in float32r (full PE rate) producing out^T,
    with the LoRA up-proj matmul fused into the same PSUM accumulation.
  - Host gathers per-core out^T shards and transposes back.
"""

import numpy as np

import concourse.bacc as bacc
import concourse.bass as bass
import concourse.mybir as mybir
import concourse.tile as tile
from concourse.bass_utils import run_bass_kernel_spmd
from concourse.masks import make_identity

F32 = mybir.dt.float32
F32R = mybir.dt.float32r
AX = mybir.AxisListType
ALU = mybir.AluOpType
ACT = mybir.ActivationFunctionType

B, S, IN, OUT = 4, 2048, 4096, 4096
E, K, R = 8, 2, 16
CAP_FACTOR = 3.0
ALPHA = 1.0 / R
LN_EPS = 1e-5
N_CORES = 8
N_TOK = B * S               # 8192
TPC = N_TOK // N_CORES      # 1024 tokens per core
G4E = 4 * E                 # 32 gate hidden
ER = E * R                  # 128
KT = IN // 128              # 32 contraction tiles
NEG = -1.0e30
CAPACITY = float(int(CAP_FACTOR * N_TOK / E))  # 3072


def _r(ap):
    """View an fp32 AP as float32r for full-rate PE matmuls."""
    return ap.bitcast(F32R)


def build_bass():
    nc = bacc.Bacc(
        "TRN2", target_bir_lowering=False, debug=False, num_devices=N_CORES
    )
    xT = nc.dram_tensor("xT", [IN, TPC], F32, kind="ExternalInput")
    wT = nc.dram_tensor("wT", [IN, OUT], F32, kind="ExternalInput")
    a_cat = nc.dram_tensor("a_cat", [IN, ER], F32, kind="ExternalInput")
    b_cat = nc.dram_tensor("b_cat", [ER, OUT], F32, kind="ExternalInput")
    g1T = nc.dram_tensor("g1T", [IN, G4E], F32, kind="ExternalInput")
    g2T = nc.dram_tensor("g2T", [G4E, E], F32, kind="ExternalInput")
    gb1r = nc.dram_tensor("gb1r", [128, G4E], F32, kind="ExternalInput")
    gamr = nc.dram_tensor("gamr", [128, G4E], F32, kind="ExternalInput")
    betr = nc.dram_tensor("betr", [128, G4E], F32, kind="ExternalInput")
    gb2r = nc.dram_tensor("gb2r", [128, E], F32, kind="ExternalInput")
    repm = nc.dram_tensor("repm", [E, ER], F32, kind="ExternalInput")
    outT = nc.dram_tensor("outT", [OUT, TPC], F32, kind="ExternalOutput")

    TB = TPC // 128  # 8 token blocks of 128
    TH = TPC // 512  # 2 token halves of 512

    with tile.TileContext(nc) as tc:
        with (
            tc.tile_pool(name="big", bufs=1) as big,
            tc.tile_pool(name="consts", bufs=1) as consts,
            tc.tile_pool(name="gate", bufs=2) as gp,
            tc.tile_pool(name="route", bufs=TB) as rp,
            tc.tile_pool(name="psum_s", bufs=3, space="PSUM") as pss,
            tc.tile_pool(name="psum_c", bufs=1, space="PSUM") as psc,
            tc.tile_pool(name="psum_m", bufs=3, space="PSUM") as psm,
            tc.tile_pool(name="wslab", bufs=2) as wsp,
            tc.tile_pool(name="outp", bufs=2) as op_,
            tc.tile_pool(name="dram", bufs=1, space="DRAM") as dp,
        ):
            # ---- resident loads -------------------------------------------
            xT_sb = big.tile([128, KT, TPC], F32R)
            xT_r = xT.ap().bitcast(F32R).rearrange("(k p) t -> k p t", p=128)
            for k in range(KT):
                nc.sync.dma_start(xT_sb[:, k], xT_r[k])
            b_sb = big.tile([128, OUT], F32R)
            nc.sync.dma_start(b_sb, b_cat.ap().bitcast(F32R))
            g1T_sb = consts.tile([128, KT, G4E], F32)
            nc.sync.dma_start(
                g1T_sb, g1T.ap().rearrange("(k p) g -> p k g", p=128)
            )
            g2T_sb = consts.tile([G4E, E], F32)
            nc.sync.dma_start(g2T_sb, g2T.ap())
            gb1_sb = consts.tile([128, G4E], F32)
            nc.sync.dma_start(gb1_sb, gb1r.ap())
            gam_sb = consts.tile([128, G4E], F32)
            nc.sync.dma_start(gam_sb, gamr.ap())
            bet_sb = consts.tile([128, G4E], F32)
            nc.sync.dma_start(bet_sb, betr.ap())
            gb2_sb = consts.tile([128, E], F32)
            nc.sync.dma_start(gb2_sb, gb2r.ap())
            rep_sb = consts.tile([E, ER], F32)
            nc.sync.dma_start(rep_sb, repm.ap())
            ident = consts.tile([128, 128], F32)
            make_identity(nc, ident)
            ones = consts.tile([128, 128], F32)
            nc.vector.memset(ones, 1.0)
            eps_sb = consts.tile([128, 1], F32)
            nc.vector.memset(eps_sb, LN_EPS)

            # ---- gate MLP + routing ---------------------------------------
            w_blk = []   # per-token-block combine pieces
            cnt_ps0 = psc.tile([1, E], F32)
            cnt_ps1 = psc.tile([1, E], F32)
            for tb in range(TB):
                ph = pss.tile([128, G4E], F32, tag="sm")
                for k in range(KT):
                    nc.tensor.matmul(
                        ph,
                        xT_sb[:, k, tb * 128 : (tb + 1) * 128].bitcast(F32),
                        g1T_sb[:, k].bitcast(F32),
                        start=(k == 0),
                        stop=(k == KT - 1),
                    )
                h = gp.tile([128, G4E], F32, tag="h")
                nc.vector.tensor_tensor(out=h, in0=ph, in1=gb1_sb, op=ALU.add)
                mu = gp.tile([128, 1], F32, tag="mu")
                nc.vector.tensor_reduce(out=mu, in_=h, axis=AX.X, op=ALU.add)
                nc.vector.tensor_scalar_mul(mu, mu, 1.0 / G4E)
                d = gp.tile([128, G4E], F32, tag="d")
                nc.vector.tensor_scalar_sub(d, h, mu)
                sq = gp.tile([128, G4E], F32, tag="sq")
                nc.vector.tensor_tensor(out=sq, in0=d, in1=d, op=ALU.mult)
                var = gp.tile([128, 1], F32, tag="var")
                nc.vector.tensor_reduce(out=var, in_=sq, axis=AX.X, op=ALU.add)
                std = gp.tile([128, 1], F32, tag="std")
                nc.scalar.activation(
                    std, var, ACT.Sqrt, bias=eps_sb[:, :], scale=1.0 / G4E
                )
                rstd = gp.tile([128, 1], F32, tag="rstd")
                nc.vector.reciprocal(rstd, std)
                hn = gp.tile([128, G4E], F32, tag="hn")
                nc.vector.tensor_scalar_mul(hn, d, rstd)
                nc.vector.tensor_tensor(out=hn, in0=hn, in1=gam_sb, op=ALU.mult)
                nc.vector.tensor_tensor(out=hn, in0=hn, in1=bet_sb, op=ALU.add)
                nc.vector.tensor_scalar_max(hn, hn, 0.0)
                # transpose h block -> [32, 128]
                pt = pss.tile([G4E, 128], F32, tag="sm")
                nc.tensor.transpose(pt, hn, ident)
                ht_t = gp.tile([G4E, 128], F32, tag="ht")
                nc.vector.tensor_copy(ht_t, pt)
                # gates for this block: [128 tok, 8]
                pg = pss.tile([128, E], F32, tag="sm")
                nc.tensor.matmul(
                    pg,
                    ht_t,
                    g2T_sb,
                    start=True,
                    stop=True,
                )
                gates = rp.tile([128, E], F32, tag="gates")
                nc.vector.tensor_tensor(out=gates, in0=pg, in1=gb2_sb, op=ALU.add)
                v1 = rp.tile([128, 1], F32, tag="v1")
                nc.vector.tensor_reduce(out=v1, in_=gates, axis=AX.X, op=ALU.max)
                oh1 = rp.tile([128, E], F32, tag="oh1")
                nc.vector.tensor_scalar(
                    out=oh1, in0=gates, scalar1=v1, scalar2=None, op0=ALU.is_ge
                )
                msk = rp.tile([128, E], F32, tag="msk")
                nc.vector.tensor_scalar_mul(msk, oh1, NEG)
                nc.vector.tensor_tensor(out=msk, in0=msk, in1=gates, op=ALU.add)
                v2 = rp.tile([128, 1], F32, tag="v2")
                nc.vector.tensor_reduce(out=v2, in_=msk, axis=AX.X, op=ALU.max)
                oh2 = rp.tile([128, E], F32, tag="oh2")
                nc.vector.tensor_scalar(
                    out=oh2, in0=msk, scalar1=v2, scalar2=None, op0=ALU.is_ge
                )
                d12 = rp.tile([128, 1], F32, tag="d12")
                nc.vector.tensor_tensor(out=d12, in0=v1, in1=v2, op=ALU.subtract)
                s1 = rp.tile([128, 1], F32, tag="s1")
                nc.scalar.activation(s1, d12, ACT.Sigmoid)
                s2 = rp.tile([128, 1], F32, tag="s2")
                nc.vector.tensor_scalar(
                    out=s2, in0=s1, scalar1=-1.0, scalar2=1.0, op0=ALU.mult, op1=ALU.add
                )
                w_blk.append((oh1, oh2, s1, s2))
                # expert counts per slot (sum over tokens via ones-matmul)
                nc.tensor.matmul(
                    cnt_ps0, ones[:, 0:1], oh1,
                    start=(tb == 0), stop=(tb == TB - 1),
                )
                nc.tensor.matmul(
                    cnt_ps1, ones[:, 0:1], oh2,
                    start=(tb == 0), stop=(tb == TB - 1),
                )

            # ---- LoRA down-proj raw: tmp[er, tok] = A_cat^T x ------------
            tmp_sb = consts.tile([128, TPC], F32)
            ptmp = [
                psm.tile([128, 512], F32, tag="big", name=f"ptmp{_i}")
                for _i in range(TH)
            ]
            for k in range(KT):
                a_t = wsp.tile([128, ER], F32R, tag="a_t")
                nc.sync.dma_start(
                    a_t, a_cat.ap().bitcast(F32R)[k * 128 : (k + 1) * 128, :]
                )
                for th in range(TH):
                    nc.tensor.matmul(
                        ptmp[th],
                        a_t,
                        xT_sb[:, k, th * 512 : (th + 1) * 512],
                        start=(k == 0),
                        stop=(k == KT - 1),
                    )
            for th in range(TH):
                nc.vector.tensor_copy(
                    tmp_sb[:, th * 512 : (th + 1) * 512], ptmp[th]
                )

            # ---- global capacity rule (AllReduce of counts) ---------------
            cnt_sb = consts.tile([1, 2 * E], F32)
            nc.vector.tensor_copy(cnt_sb[:, 0:E], cnt_ps0)
            nc.vector.tensor_copy(cnt_sb[:, E : 2 * E], cnt_ps1)
            cc_in = dp.tile([1, 2 * E], F32)
            cc_out = dp.tile([1, 2 * E], F32)
            nc.sync.dma_start(cc_in, cnt_sb)
            nc.gpsimd.collective_compute(
                "AllReduce",
                ALU.add,
                replica_groups=[list(range(N_CORES))],
                ins=[cc_in.opt()],
                outs=[cc_out.opt()],
            )
            cntg_sb = consts.tile([1, 2 * E], F32)
            nc.sync.dma_start(cntg_sb, cc_out)
            alw1 = consts.tile([1, 2 * E], F32)
            nc.vector.tensor_scalar(
                out=alw1, in0=cntg_sb, scalar1=CAPACITY + 0.5, scalar2=None,
                op0=ALU.is_le,
            )
            pb = pss.tile([128, 2 * E], F32, tag="sm")
            nc.tensor.matmul(pb, ones[0:1, :], alw1, start=True, stop=True)
            alw = consts.tile([128, 2 * E], F32)
            nc.vector.tensor_copy(alw, pb)

            # combine weights w[tok, e], then transpose to [e, tok]
            wT_sb = consts.tile([E, TPC], F32)
            for tb in range(TB):
                oh1, oh2, s1, s2 = w_blk[tb]
                t1 = gp.tile([128, E], F32, tag="t1")
                nc.vector.tensor_tensor(out=t1, in0=oh1, in1=alw[:, 0:E], op=ALU.mult)
                nc.vector.tensor_scalar_mul(t1, t1, s1)
                t2 = gp.tile([128, E], F32, tag="t2")
                nc.vector.tensor_tensor(
                    out=t2, in0=oh2, in1=alw[:, E : 2 * E], op=ALU.mult
                )
                nc.vector.tensor_scalar_mul(t2, t2, s2)
                nc.vector.tensor_tensor(out=t1, in0=t1, in1=t2, op=ALU.add)
                ptw = pss.tile([E, 128], F32, tag="sm")
                nc.tensor.transpose(ptw, t1, ident)
                nc.vector.tensor_copy(wT_sb[:, tb * 128 : (tb + 1) * 128], ptw)

            # broadcast over rank dim: wbr[e*16+r, tok]
            wbr_sb = consts.tile([128, TPC], F32)
            for th in range(TH):
                pwb = psm.tile([128, 512], F32, tag="big")
                nc.tensor.matmul(
                    pwb, rep_sb, wT_sb[:, th * 512 : (th + 1) * 512],
                    start=True, stop=True,
                )
                nc.vector.tensor_copy(wbr_sb[:, th * 512 : (th + 1) * 512], pwb)

            tw_sb = consts.tile([128, TPC], F32R)
            for th in range(TH):
                nc.vector.tensor_tensor(
                    out=tw_sb[:, th * 512 : (th + 1) * 512],
                    in0=tmp_sb[:, th * 512 : (th + 1) * 512],
                    in1=wbr_sb[:, th * 512 : (th + 1) * 512],
                    op=ALU.mult,
                )

            # ---- main matmul (W stationary, x moving) + fused LoRA-B ------
            for oc in range(OUT // 128):
                wsl = wsp.tile([128, KT, 128], F32R, tag="wsl")
                nc.sync.dma_start(
                    wsl,
                    wT.ap().bitcast(F32R)[:, oc * 128 : (oc + 1) * 128].rearrange(
                        "(k p) c -> p k c", p=128
                    ),
                )
                for th in range(TH):
                    po = psm.tile([128, 512], F32, tag="big")
                    for k in range(KT):
                        nc.tensor.matmul(
                            po,
                            wsl[:, k],
                            xT_sb[:, k, th * 512 : (th + 1) * 512],
                            start=(k == 0),
                            stop=False,
                        )
                    nc.tensor.matmul(
                        po,
                        b_sb[:, oc * 128 : (oc + 1) * 128],
                        tw_sb[:, th * 512 : (th + 1) * 512],
                        start=False,
                        stop=True,
                    )
                    osb = op_.tile([128, 512], F32, tag="osb")
                    nc.vector.tensor_copy(osb, po)
                    nc.sync.dma_start(
                        outT.ap()[
                            oc * 128 : (oc + 1) * 128, th * 512 : (th + 1) * 512
                        ],
                        osb,
                    )
    return nc


_CACHE = {}


def _get_nc():
    if "nc" not in _CACHE:
        nc = build_bass()
        nc.finalize()
        _CACHE["nc"] = nc
    return _CACHE["nc"]


def prep_in_maps(inputs):
    x = np.asarray(inputs["x"], dtype=np.float32)
    weight = np.asarray(inputs["weight"], dtype=np.float32)
    xf = x.reshape(N_TOK, IN)
    wT = np.ascontiguousarray(weight.T)
    a_cat = np.ascontiguousarray(
        np.asarray(inputs["lora_A"], np.float32).transpose(1, 0, 2).reshape(IN, ER)
        * ALPHA
    )
    b_cat = np.ascontiguousarray(
        np.asarray(inputs["lora_B"], np.float32).reshape(ER, OUT)
    )
    g1T = np.ascontiguousarray(np.asarray(inputs["gw1"], np.float32).T)
    g2T = np.ascontiguousarray(np.asarray(inputs["gw2"], np.float32).T)
    gb1r = np.ascontiguousarray(
        np.broadcast_to(np.asarray(inputs["gb1"], np.float32), (128, G4E))
    )
    gamr = np.ascontiguousarray(
        np.broadcast_to(np.asarray(inputs["ln_gamma"], np.float32), (128, G4E))
    )
    betr = np.ascontiguousarray(
        np.broadcast_to(np.asarray(inputs["ln_beta"], np.float32), (128, G4E))
    )
    gb2r = np.ascontiguousarray(
        np.broadcast_to(np.asarray(inputs["gb2"], np.float32), (128, E))
    )
    repm = np.zeros((E, ER), np.float32)
    for e in range(E):
        repm[e, e * R : (e + 1) * R] = 1.0

    shared = dict(
        wT=wT, a_cat=a_cat, b_cat=b_cat, g1T=g1T, g2T=g2T,
        gb1r=gb1r, gamr=gamr, betr=betr, gb2r=gb2r, repm=repm,
    )
    in_maps = []
    for c in range(N_CORES):
        xT_c = np.ascontiguousarray(xf[c * TPC : (c + 1) * TPC].T)
        in_maps.append(dict(xT=xT_c, **shared))
    return in_maps


def gather(results):
    out = np.empty((N_TOK, OUT), np.float32)
    for c in range(N_CORES):
        out[c * TPC : (c + 1) * TPC] = results[c]["outT"].T
    return out.reshape(B, S, OUT)


def kernel(**inputs):
    in_maps = prep_in_maps(inputs)
    nc = _get_nc()
    res = run_bass_kernel_spmd(nc, in_maps, core_ids=list(range(N_CORES)))
    return gather(res.results)



# revision 2
# speedup vs baseline: 1.2408x; 1.2408x over previous
"""LoRA-MoE Linear kernel for Trainium2, 8-core SPMD.

Strategy (token-parallel, bf16 compute):
  - 8192 tokens sharded across 8 cores (1024 each). Host pre-lays-out every
    large operand so each DMA is contiguous per partition (128 x 8KB
    descriptors per weight slab instead of 4096 x 512B).
  - All large matmuls run in bf16 (full PE rate + fast weight load); PSUM
    accumulates fp32. Tolerance is 2e-2 L2, bf16 contributes ~3e-3.
  - Pass 1 (channel-major): tmp[er,tok] = A^T x and h[32,tok] = G1^T x share
    the same N=512 moving-x k-loop (128 MMs total, vs 256 tiny fp32 MMs).
  - Routing stays expert-major [8, tok]: partition_all_reduce (GpSimd) gives
    channel means / top-2 maxima without any PE transposes. All elementwise
    work that depends on the counts AllReduce runs on GpSimd so the Vector
    queue can never head-of-line block on the collective.
  - Main matmul (W-stationary per 128-col block, x moving) depends only on
    x + weights, so it streams PE-saturating MMs from ~2us onward. The
    routed LoRA-B pass is emitted last in program order and written to a
    separate DRAM tensor; the host adds main+lora during gather. This keeps
    every collective-dependent PE instruction at the tail of the strict-FIFO
    PE queue.
"""

import numpy as np
import ml_dtypes

import concourse.bacc as bacc
import concourse.bass as bass
import concourse.bass_isa as bass_isa
import concourse.mybir as mybir
import concourse.tile as tile
from concourse.bass_utils import run_bass_kernel_spmd

F32 = mybir.dt.float32
BF16 = mybir.dt.bfloat16
AX = mybir.AxisListType
ALU = mybir.AluOpType
ACT = mybir.ActivationFunctionType
RED = bass_isa.ReduceOp
BF = ml_dtypes.bfloat16

B, S, IN, OUT = 4, 2048, 4096, 4096
E, K, R = 8, 2, 16
CAP_FACTOR = 3.0
ALPHA = 1.0 / R
LN_EPS = 1e-5
N_CORES = 8
N_TOK = B * S               # 8192
TPC = N_TOK // N_CORES      # 1024 tokens per core
G4E = 4 * E                 # 32 gate hidden
ER = E * R                  # 128
KT = IN // 128              # 32 contraction tiles
OC = OUT // 128             # 32 output column blocks
NEG = -1.0e30
CAPACITY = float(int(CAP_FACTOR * N_TOK / E))  # 3072
TH = TPC // 512             # 2 token halves of 512


def build_bass():
    nc = bacc.Bacc(
        "TRN2", target_bir_lowering=False, debug=False, num_devices=N_CORES
    )
    xp = nc.dram_tensor("xp", [128, KT * TPC], BF16, kind="ExternalInput")
    wp = nc.dram_tensor("wp", [OC, 128, KT * 128], BF16, kind="ExternalInput")
    ap_ = nc.dram_tensor("ap_", [128, KT * ER], BF16, kind="ExternalInput")
    g1p = nc.dram_tensor("g1p", [128, KT * G4E], BF16, kind="ExternalInput")
    bp = nc.dram_tensor("bp", [ER, OUT], BF16, kind="ExternalInput")
    g2p = nc.dram_tensor("g2p", [G4E, E], BF16, kind="ExternalInput")
    repp = nc.dram_tensor("repp", [E, ER], BF16, kind="ExternalInput")
    gb1c = nc.dram_tensor("gb1c", [G4E, 1], F32, kind="ExternalInput")
    gamc = nc.dram_tensor("gamc", [G4E, 1], F32, kind="ExternalInput")
    betc = nc.dram_tensor("betc", [G4E, 1], F32, kind="ExternalInput")
    gb2c = nc.dram_tensor("gb2c", [E, 1], F32, kind="ExternalInput")
    outT = nc.dram_tensor("outT", [OUT, TPC], F32, kind="ExternalOutput")
    loraT = nc.dram_tensor("loraT", [OUT, TPC], F32, kind="ExternalOutput")

    with tile.TileContext(nc) as tc:
        with (
            tc.tile_pool(name="big", bufs=1) as big,
            tc.tile_pool(name="rt", bufs=1) as rt,
            tc.tile_pool(name="wsl", bufs=3) as wsp,
            tc.tile_pool(name="outp", bufs=4) as op_,
            tc.tile_pool(name="ps_a", bufs=1, space="PSUM") as psa,
            tc.tile_pool(name="ps_c", bufs=2, space="PSUM") as psc,
            tc.tile_pool(name="ps_m", bufs=4, space="PSUM") as psm,
            tc.tile_pool(name="dram", bufs=1, space="DRAM") as dp,
        ):
            # ---- resident loads (x first: it gates everything) ------------
            xT_sb = big.tile([128, KT, TPC], BF16)
            for k in range(KT):
                nc.sync.dma_start(
                    xT_sb[:, k], xp.ap()[:, k * TPC : (k + 1) * TPC]
                )
            a_sb = big.tile([128, KT, ER], BF16)
            nc.sync.dma_start(a_sb, ap_.ap().rearrange("p (k e) -> p k e", e=ER))
            g1_sb = big.tile([128, KT, G4E], BF16)
            nc.sync.dma_start(
                g1_sb, g1p.ap().rearrange("p (k g) -> p k g", g=G4E)
            )
            b_sb = big.tile([ER, OUT], BF16)
            nc.sync.dma_start(b_sb, bp.ap())
            g2_sb = big.tile([G4E, E], BF16)
            nc.sync.dma_start(g2_sb, g2p.ap())
            repp_sb = big.tile([E, ER], BF16)
            nc.sync.dma_start(repp_sb, repp.ap())
            gb1c_sb = big.tile([G4E, 1], F32)
            nc.sync.dma_start(gb1c_sb, gb1c.ap())
            gamc_sb = big.tile([G4E, 1], F32)
            nc.sync.dma_start(gamc_sb, gamc.ap())
            betc_sb = big.tile([G4E, 1], F32)
            nc.sync.dma_start(betc_sb, betc.ap())
            gb2c_sb = big.tile([E, 1], F32)
            nc.sync.dma_start(gb2c_sb, gb2c.ap())
            eps_sb = big.tile([G4E, 1], F32)
            nc.vector.memset(eps_sb, LN_EPS)

            # ---- pass 1: tmp[er,tok] = A^T x, hT[32,tok] = G1^T x ---------
            tmp_ps = [psa.tile([128, 512], F32, name=f"tmp{t}") for t in range(TH)]
            hT_ps = [psc.tile([G4E, 512], F32, tag="sm", name=f"hT{t}") for t in range(TH)]
            for k in range(KT):
                for th in range(TH):
                    xm = xT_sb[:, k, th * 512 : (th + 1) * 512]
                    nc.tensor.matmul(
                        tmp_ps[th], a_sb[:, k], xm,
                        start=(k == 0), stop=(k == KT - 1),
                    )
                    nc.tensor.matmul(
                        hT_ps[th], g1_sb[:, k], xm,
                        start=(k == 0), stop=(k == KT - 1),
                    )
            tmp_sb = big.tile([128, TPC], F32)
            h_sb = big.tile([G4E, TPC], F32)
            for th in range(TH):
                sl = slice(th * 512, (th + 1) * 512)
                nc.vector.tensor_copy(tmp_sb[:, sl], tmp_ps[th])
                nc.vector.tensor_scalar(
                    out=h_sb[:, sl], in0=hT_ps[th], scalar1=gb1c_sb,
                    scalar2=None, op0=ALU.add,
                )

            # ---- LayerNorm over the 32 gate channels (partition axis) ----
            mu = rt.tile([G4E, TPC], F32, tag="mu")
            nc.gpsimd.partition_all_reduce(mu, h_sb, channels=G4E, reduce_op=RED.add)
            nc.vector.tensor_scalar_mul(mu, mu, 1.0 / G4E)
            nc.vector.tensor_tensor(out=h_sb, in0=h_sb, in1=mu, op=ALU.subtract)
            sq = rt.tile([G4E, TPC], F32, tag="sq")
            nc.vector.tensor_tensor(out=sq, in0=h_sb, in1=h_sb, op=ALU.mult)
            varb = rt.tile([G4E, TPC], F32, tag="varb")
            nc.gpsimd.partition_all_reduce(varb, sq, channels=G4E, reduce_op=RED.add)
            rstd = rt.tile([G4E, TPC], F32, tag="rstd")
            nc.scalar.activation(
                rstd, varb, ACT.Sqrt, bias=eps_sb[:, :], scale=1.0 / G4E
            )
            nc.vector.reciprocal(rstd, rstd)
            nc.vector.tensor_tensor(out=h_sb, in0=h_sb, in1=rstd, op=ALU.mult)
            nc.vector.tensor_scalar(
                out=h_sb, in0=h_sb, scalar1=gamc_sb, scalar2=None, op0=ALU.mult
            )
            nc.vector.tensor_scalar(
                out=h_sb, in0=h_sb, scalar1=betc_sb, scalar2=None, op0=ALU.add
            )
            hn_bf = big.tile([G4E, TPC], BF16)
            nc.vector.tensor_scalar_max(hn_bf, h_sb, 0.0)

            # ---- gates[8,tok] = G2^T hn + b2 ------------------------------
            gates = rt.tile([E, TPC], F32, tag="gates")
            for th in range(TH):
                sl = slice(th * 512, (th + 1) * 512)
                g_ps = psc.tile([E, 512], F32, tag="sm", name=f"g{th}")
                nc.tensor.matmul(g_ps, g2_sb, hn_bf[:, sl], start=True, stop=True)
                nc.vector.tensor_scalar(
                    out=gates[:, sl], in0=g_ps, scalar1=gb2c_sb,
                    scalar2=None, op0=ALU.add,
                )

            # ---- top-2 routing, expert-major ------------------------------
            v1 = rt.tile([E, TPC], F32, tag="v1")
            nc.gpsimd.partition_all_reduce(v1, gates, channels=E, reduce_op=RED.max)
            oh1 = rt.tile([E, TPC], F32, tag="oh1")
            nc.vector.tensor_tensor(out=oh1, in0=gates, in1=v1, op=ALU.is_ge)
            msk = rt.tile([E, TPC], F32, tag="msk")
            nc.vector.tensor_scalar_mul(msk, oh1, NEG)
            nc.vector.tensor_tensor(out=msk, in0=msk, in1=gates, op=ALU.add)
            v2 = rt.tile([E, TPC], F32, tag="v2")
            nc.gpsimd.partition_all_reduce(v2, msk, channels=E, reduce_op=RED.max)
            oh2 = rt.tile([E, TPC], F32, tag="oh2")
            nc.vector.tensor_tensor(out=oh2, in0=msk, in1=v2, op=ALU.is_ge)
            nc.vector.tensor_tensor(out=msk, in0=v1, in1=v2, op=ALU.subtract)
            s1 = rt.tile([E, TPC], F32, tag="s1")
            nc.scalar.activation(s1, msk, ACT.Sigmoid)
            s2 = rt.tile([E, TPC], F32, tag="s2")
            nc.vector.tensor_scalar(
                out=s2, in0=s1, scalar1=-1.0, scalar2=1.0, op0=ALU.mult, op1=ALU.add
            )

            # ---- global capacity rule (tiny AllReduce of counts) ----------
            cnt = rt.tile([E, 2], F32, tag="cnt")
            nc.vector.tensor_reduce(out=cnt[:, 0:1], in_=oh1, axis=AX.X, op=ALU.add)
            nc.vector.tensor_reduce(out=cnt[:, 1:2], in_=oh2, axis=AX.X, op=ALU.add)
            cc_in = dp.tile([E, 2], F32)
            cc_out = dp.tile([E, 2], F32)
            nc.gpsimd.dma_start(cc_in, cnt)
            nc.gpsimd.collective_compute(
                "AllReduce",
                ALU.add,
                replica_groups=[list(range(N_CORES))],
                ins=[cc_in.opt()],
                outs=[cc_out.opt()],
            )
            cntg = rt.tile([E, 2], F32, tag="cntg")
            nc.gpsimd.dma_start(cntg, cc_out)
            # everything downstream of the collective runs on GpSimd: the
            # Vector queue must never wait on cross-core progress.
            alw = rt.tile([E, 2], F32, tag="alw")
            nc.gpsimd.tensor_scalar(
                out=alw, in0=cntg, scalar1=CAPACITY + 0.5, scalar2=None,
                op0=ALU.is_le,
            )
            w1 = rt.tile([E, TPC], F32, tag="w1")
            nc.gpsimd.tensor_scalar(
                out=w1, in0=oh1, scalar1=alw[:, 0:1], scalar2=None, op0=ALU.mult
            )
            nc.gpsimd.tensor_tensor(out=w1, in0=w1, in1=s1, op=ALU.mult)
            w2 = rt.tile([E, TPC], F32, tag="w2")
            nc.gpsimd.tensor_scalar(
                out=w2, in0=oh2, scalar1=alw[:, 1:2], scalar2=None, op0=ALU.mult
            )
            nc.gpsimd.tensor_tensor(out=w2, in0=w2, in1=s2, op=ALU.mult)
            nc.gpsimd.tensor_tensor(out=w1, in0=w1, in1=w2, op=ALU.add)
            wT_bf = big.tile([E, TPC], BF16)
            nc.gpsimd.tensor_copy(wT_bf, w1)

            # ---- main matmul: depends only on x and W ---------------------
            for oc in range(OC):
                wsl = wsp.tile([128, KT, 128], BF16, tag="wsl")
                nc.sync.dma_start(
                    wsl, wp.ap()[oc].rearrange("p (k c) -> p k c", c=128)
                )
                pos = [
                    psm.tile([128, 512], F32, tag="po", name=f"po{oc}_{t}")
                    for t in range(TH)
                ]
                for k in range(KT):
                    for th in range(TH):
                        nc.tensor.matmul(
                            pos[th],
                            wsl[:, k],
                            xT_sb[:, k, th * 512 : (th + 1) * 512],
                            start=(k == 0),
                            stop=(k == KT - 1),
                        )
                for th in range(TH):
                    osb = op_.tile([128, 512], F32, tag="osb")
                    nc.vector.tensor_copy(osb, pos[th])
                    nc.sync.dma_start(
                        outT.ap()[
                            oc * 128 : (oc + 1) * 128, th * 512 : (th + 1) * 512
                        ],
                        osb,
                    )

            # ---- routed LoRA-B pass (tail of the PE stream) ---------------
            tw_bf = big.tile([128, TPC], BF16)
            for th in range(TH):
                sl = slice(th * 512, (th + 1) * 512)
                wbr = psc.tile([128, 512], F32, tag="sm", name=f"wbr{th}")
                nc.tensor.matmul(wbr, repp_sb, wT_bf[:, sl], start=True, stop=True)
                nc.vector.tensor_tensor(
                    out=tw_bf[:, sl], in0=tmp_sb[:, sl], in1=wbr, op=ALU.mult
                )
            for oc in range(OC):
                for th in range(TH):
                    lp = psc.tile([128, 512], F32, tag="sm", name=f"lp{oc}_{th}")
                    nc.tensor.matmul(
                        lp,
                        b_sb[:, oc * 128 : (oc + 1) * 128],
                        tw_bf[:, th * 512 : (th + 1) * 512],
                        start=True,
                        stop=True,
                    )
                    lsb = op_.tile([128, 512], F32, tag="lsb")
                    nc.vector.tensor_copy(lsb, lp)
                    nc.sync.dma_start(
                        loraT.ap()[
                            oc * 128 : (oc + 1) * 128, th * 512 : (th + 1) * 512
                        ],
                        lsb,
                    )
    return nc


_CACHE = {}


def _get_nc():
    if "nc" not in _CACHE:
        nc = build_bass()
        nc.finalize()
        _CACHE["nc"] = nc
    return _CACHE["nc"]


def prep_in_maps(inputs):
    x = np.asarray(inputs["x"], dtype=np.float32)
    weight = np.asarray(inputs["weight"], dtype=np.float32)
    xf = x.reshape(N_TOK, IN)
    # wp[oc, p, k*128+c] = weight[oc*128+c, k*128+p]
    wp = np.ascontiguousarray(
        weight.reshape(OC, 128, KT, 128).transpose(0, 3, 2, 1).reshape(OC, 128, KT * 128)
    ).astype(BF)
    a_cat = (
        np.asarray(inputs["lora_A"], np.float32).transpose(1, 0, 2).reshape(IN, ER)
        * ALPHA
    )
    ap_ = np.ascontiguousarray(
        a_cat.reshape(KT, 128, ER).transpose(1, 0, 2).reshape(128, KT * ER)
    ).astype(BF)
    g1T = np.asarray(inputs["gw1"], np.float32).T  # [IN, 32]
    g1p = np.ascontiguousarray(
        g1T.reshape(KT, 128, G4E).transpose(1, 0, 2).reshape(128, KT * G4E)
    ).astype(BF)
    bp = np.asarray(inputs["lora_B"], np.float32).reshape(ER, OUT).astype(BF)
    g2p = np.ascontiguousarray(np.asarray(inputs["gw2"], np.float32).T).astype(BF)
    repm = np.zeros((E, ER), np.float32)
    for e in range(E):
        repm[e, e * R : (e + 1) * R] = 1.0
    repp = repm.astype(BF)
    gb1c = np.ascontiguousarray(np.asarray(inputs["gb1"], np.float32).reshape(G4E, 1))
    gamc = np.ascontiguousarray(
        np.asarray(inputs["ln_gamma"], np.float32).reshape(G4E, 1)
    )
    betc = np.ascontiguousarray(
        np.asarray(inputs["ln_beta"], np.float32).reshape(G4E, 1)
    )
    gb2c = np.ascontiguousarray(np.asarray(inputs["gb2"], np.float32).reshape(E, 1))

    shared = dict(
        wp=wp, ap_=ap_, g1p=g1p, bp=bp, g2p=g2p, repp=repp,
        gb1c=gb1c, gamc=gamc, betc=betc, gb2c=gb2c,
    )
    in_maps = []
    for c in range(N_CORES):
        xs = xf[c * TPC : (c + 1) * TPC]  # [TPC, IN]
        xpc = np.ascontiguousarray(
            xs.T.reshape(KT, 128, TPC).transpose(1, 0, 2).reshape(128, KT * TPC)
        ).astype(BF)
        in_maps.append(dict(xp=xpc, **shared))
    return in_maps


def gather(results):
    out = np.empty((N_TOK, OUT), np.float32)
    for c in range(N_CORES):
        tot = results[c]["outT"] + results[c]["loraT"]
        out[c * TPC : (c + 1) * TPC] = tot.T
    return out.reshape(B, S, OUT)


def kernel(**inputs):
    in_maps = prep_in_maps(inputs)
    nc = _get_nc()
    res = run_bass_kernel_spmd(nc, in_maps, core_ids=list(range(N_CORES)))
    return gather(res.results)


# revision 5
# speedup vs baseline: 1.3284x; 1.0706x over previous
"""LoRA-MoE Linear kernel for Trainium2, 8-core SPMD.

Strategy (token-parallel, bf16 compute):
  - 8192 tokens sharded across 8 cores (1024 each). Host pre-lays-out every
    large operand so each DMA is contiguous per partition.
  - All large matmuls in bf16 (full PE rate); PSUM accumulates fp32.
  - Pass 1 (channel-major): tmp[er,tok] = A^T x and d[32,tok] = (G1-mean)^T x
    share one N=512 moving-x k-loop. The LayerNorm mean subtraction is folded
    into centered gate weights host-side, so PSUM directly yields d.
  - Routing stays expert-major [8,tok]: partition_all_reduce (GpSimd) for
    var / top-2 maxima; zero PE transposes.
  - Engine queues are strict FIFO per engine, so placement = program order:
      PE:     pass1 | oc0 | gates | oc1..oc11 (unfused) | wbr |
              oc12..oc31 (lora fused into the PSUM accumulation) | lora tail
      Vector: pass1 copies, LN, pre-collective routing, post-collective
              combine, tw (nothing after => can't block anything)
      Scalar: Rsqrt, Sigmoid, ALL PSUM->SBUF output copies (ACT.Copy)
      GpSimd: partition reductions, counts-AllReduce + its DMAs
    The counts AllReduce finishes ~160-220us (mesh hop DMAs queue behind
    weight-slab traffic), so LoRA is only fused for oc>=12; oc<12 get a
    separate LoRA pass at the tail written to loraT, host adds.
"""

import numpy as np
import ml_dtypes

import concourse.bacc as bacc
import concourse.bass as bass
import concourse.bass_isa as bass_isa
import concourse.mybir as mybir
import concourse.tile as tile
from concourse.bass_utils import run_bass_kernel_spmd

F32 = mybir.dt.float32
BF16 = mybir.dt.bfloat16
AX = mybir.AxisListType
ALU = mybir.AluOpType
ACT = mybir.ActivationFunctionType
RED = bass_isa.ReduceOp
BF = ml_dtypes.bfloat16

B, S, IN, OUT = 4, 2048, 4096, 4096
E, K, R = 8, 2, 16
CAP_FACTOR = 3.0
ALPHA = 1.0 / R
LN_EPS = 1e-5
N_CORES = 8
N_TOK = B * S               # 8192
TPC = N_TOK // N_CORES      # 1024 tokens per core
G4E = 4 * E                 # 32 gate hidden
ER = E * R                  # 128
KT = IN // 128              # 32 contraction tiles
OC = OUT // 128             # 32 output column blocks
NEG = -1.0e30
CAPACITY = float(int(CAP_FACTOR * N_TOK / E))  # 3072
TH = TPC // 512             # 2 token halves of 512
FUSE_OC = 12                # oc >= FUSE_OC get LoRA fused into main PSUM


def build_bass():
    nc = bacc.Bacc(
        "TRN2", target_bir_lowering=False, debug=False, num_devices=N_CORES
    )
    xp = nc.dram_tensor("xp", [128, KT * TPC], BF16, kind="ExternalInput")
    wp = nc.dram_tensor("wp", [OC, 128, KT * 128], BF16, kind="ExternalInput")
    ap_ = nc.dram_tensor("ap_", [128, KT * ER], BF16, kind="ExternalInput")
    g1p = nc.dram_tensor("g1p", [128, KT * G4E], BF16, kind="ExternalInput")
    bp = nc.dram_tensor("bp", [ER, OUT], BF16, kind="ExternalInput")
    g2p = nc.dram_tensor("g2p", [G4E, E], BF16, kind="ExternalInput")
    repp = nc.dram_tensor("repp", [E, ER], BF16, kind="ExternalInput")
    gb1c = nc.dram_tensor("gb1c", [G4E, 1], F32, kind="ExternalInput")
    gamc = nc.dram_tensor("gamc", [G4E, 1], F32, kind="ExternalInput")
    betc = nc.dram_tensor("betc", [G4E, 1], F32, kind="ExternalInput")
    gb2c = nc.dram_tensor("gb2c", [E, 1], F32, kind="ExternalInput")
    outT = nc.dram_tensor("outT", [OUT, TPC], F32, kind="ExternalOutput")
    loraT = nc.dram_tensor("loraT", [FUSE_OC * 128, TPC], F32, kind="ExternalOutput")

    with tile.TileContext(nc) as tc:
        with (
            tc.tile_pool(name="big", bufs=1) as big,
            tc.tile_pool(name="rt", bufs=1) as rt,
            tc.tile_pool(name="wsl", bufs=3) as wsp,
            tc.tile_pool(name="outp", bufs=4) as op_,
            tc.tile_pool(name="ps_a", bufs=1, space="PSUM") as psa,
            tc.tile_pool(name="ps_c", bufs=2, space="PSUM") as psc,
            tc.tile_pool(name="ps_m", bufs=4, space="PSUM") as psm,
            tc.tile_pool(name="dram", bufs=1, space="DRAM") as dp,
        ):
            # ---- resident loads (x first, 4-k-tile chunks) ----------------
            xT_sb = big.tile([128, KT, TPC], BF16)
            for c in range(KT // 4):
                nc.sync.dma_start(
                    xT_sb[:, 4 * c : 4 * c + 4],
                    xp.ap()[:, 4 * c * TPC : (4 * c + 4) * TPC].rearrange(
                        "p (k t) -> p k t", t=TPC
                    ),
                )
            a_sb = big.tile([128, KT, ER], BF16)
            nc.sync.dma_start(a_sb, ap_.ap().rearrange("p (k e) -> p k e", e=ER))
            g1_sb = big.tile([128, KT, G4E], BF16)
            nc.sync.dma_start(
                g1_sb, g1p.ap().rearrange("p (k g) -> p k g", g=G4E)
            )
            b_sb = big.tile([ER, OUT], BF16)
            nc.sync.dma_start(b_sb, bp.ap())
            g2_sb = big.tile([G4E, E], BF16)
            nc.sync.dma_start(g2_sb, g2p.ap())
            repp_sb = big.tile([E, ER], BF16)
            nc.sync.dma_start(repp_sb, repp.ap())
            gb1c_sb = big.tile([G4E, 1], F32)
            nc.sync.dma_start(gb1c_sb, gb1c.ap())
            gamc_sb = big.tile([G4E, 1], F32)
            nc.sync.dma_start(gamc_sb, gamc.ap())
            betc_sb = big.tile([G4E, 1], F32)
            nc.sync.dma_start(betc_sb, betc.ap())
            gb2c_sb = big.tile([E, 1], F32)
            nc.sync.dma_start(gb2c_sb, gb2c.ap())
            eps_sb = big.tile([G4E, 1], F32)
            nc.vector.memset(eps_sb, LN_EPS)

            # ---- pass 1: tmp[er,tok] = A^T x, d[32,tok] = (G1-mean)^T x ---
            tmp_ps = [psa.tile([128, 512], F32, name=f"tmp{t}") for t in range(TH)]
            hT_ps = [
                psc.tile([G4E, 512], F32, tag="sm", name=f"hT{t}") for t in range(TH)
            ]
            for k in range(KT):
                for th in range(TH):
                    nc.tensor.matmul(
                        tmp_ps[th], a_sb[:, k],
                        xT_sb[:, k, th * 512 : (th + 1) * 512],
                        start=(k == 0), stop=(k == KT - 1),
                    )
                for th in range(TH):
                    nc.tensor.matmul(
                        hT_ps[th], g1_sb[:, k],
                        xT_sb[:, k, th * 512 : (th + 1) * 512],
                        start=(k == 0), stop=(k == KT - 1),
                    )
            tmp_sb = big.tile([128, TPC], F32)
            d_sb = big.tile([G4E, TPC], F32)
            for th in range(TH):
                sl = slice(th * 512, (th + 1) * 512)
                nc.vector.tensor_copy(tmp_sb[:, sl], tmp_ps[th])
                # d = (G1-centered)^T x + (gb1 - mean(gb1))  [host-folded]
                nc.vector.tensor_scalar(
                    out=d_sb[:, sl], in0=hT_ps[th], scalar1=gb1c_sb,
                    scalar2=None, op0=ALU.add,
                )

            # ---- main oc0 (hides the LN chain latency) --------------------
            def main_oc(oc, fused):
                wsl = wsp.tile([128, KT, 128], BF16, tag="wsl")
                nc.sync.dma_start(
                    wsl, wp.ap()[oc].rearrange("p (k c) -> p k c", c=128)
                )
                pos = [
                    psm.tile([128, 512], F32, tag="po", name=f"po{oc}_{t}")
                    for t in range(TH)
                ]
                for k in range(KT):
                    for th in range(TH):
                        nc.tensor.matmul(
                            pos[th], wsl[:, k],
                            xT_sb[:, k, th * 512 : (th + 1) * 512],
                            start=(k == 0),
                            stop=(not fused and k == KT - 1),
                        )
                for th in range(TH):
                    sl = slice(th * 512, (th + 1) * 512)
                    if fused:
                        nc.tensor.matmul(
                            pos[th], b_sb[:, oc * 128 : (oc + 1) * 128],
                            tw_bf[:, sl], start=False, stop=True,
                        )
                    osb = op_.tile([128, 512], F32, tag="osb")
                    nc.scalar.activation(osb, pos[th], ACT.Copy)
                    nc.sync.dma_start(
                        outT.ap()[oc * 128 : (oc + 1) * 128, sl], osb
                    )

            tw_bf = big.tile([128, TPC], BF16)
            main_oc(0, False)

            # ---- LayerNorm tail + gate logits -----------------------------
            sq = rt.tile([G4E, TPC], F32, tag="sq")
            nc.vector.tensor_tensor(out=sq, in0=d_sb, in1=d_sb, op=ALU.mult)
            varb = rt.tile([G4E, TPC], F32, tag="varb")
            nc.gpsimd.partition_all_reduce(varb, sq, channels=G4E, reduce_op=RED.add)
            rstd = rt.tile([G4E, TPC], F32, tag="rstd")
            nc.scalar.activation(
                rstd, varb, ACT.Sqrt, bias=eps_sb[:, :], scale=1.0 / G4E
            )
            nc.vector.reciprocal(rstd, rstd)
            nc.vector.tensor_tensor(out=d_sb, in0=d_sb, in1=rstd, op=ALU.mult)
            nc.vector.tensor_scalar(
                out=d_sb, in0=d_sb, scalar1=gamc_sb, scalar2=None, op0=ALU.mult
            )
            nc.vector.tensor_scalar(
                out=d_sb, in0=d_sb, scalar1=betc_sb, scalar2=None, op0=ALU.add
            )
            hn_bf = big.tile([G4E, TPC], BF16)
            nc.vector.tensor_scalar_max(hn_bf, d_sb, 0.0)

            gates = rt.tile([E, TPC], F32, tag="gates")
            for th in range(TH):
                sl = slice(th * 512, (th + 1) * 512)
                g_ps = psc.tile([E, 512], F32, tag="sm", name=f"g{th}")
                nc.tensor.matmul(g_ps, g2_sb, hn_bf[:, sl], start=True, stop=True)
                nc.vector.tensor_scalar(
                    out=gates[:, sl], in0=g_ps, scalar1=gb2c_sb,
                    scalar2=None, op0=ALU.add,
                )

            # ---- top-2 routing, expert-major ------------------------------
            v1 = rt.tile([E, TPC], F32, tag="v1")
            nc.gpsimd.partition_all_reduce(v1, gates, channels=E, reduce_op=RED.max)
            oh1 = rt.tile([E, TPC], F32, tag="oh1")
            nc.vector.tensor_tensor(out=oh1, in0=gates, in1=v1, op=ALU.is_ge)
            msk = rt.tile([E, TPC], F32, tag="msk")
            nc.vector.scalar_tensor_tensor(
                out=msk, in0=oh1, scalar=NEG, in1=gates, op0=ALU.mult, op1=ALU.add
            )
            v2 = rt.tile([E, TPC], F32, tag="v2")
            nc.gpsimd.partition_all_reduce(v2, msk, channels=E, reduce_op=RED.max)
            oh2 = rt.tile([E, TPC], F32, tag="oh2")
            nc.vector.tensor_tensor(out=oh2, in0=msk, in1=v2, op=ALU.is_ge)
            nc.vector.tensor_tensor(out=msk, in0=v1, in1=v2, op=ALU.subtract)
            s1 = rt.tile([E, TPC], F32, tag="s1")
            nc.scalar.activation(s1, msk, ACT.Sigmoid)
            u1 = rt.tile([E, TPC], F32, tag="u1")
            nc.vector.tensor_tensor(out=u1, in0=oh1, in1=s1, op=ALU.mult)
            u2 = rt.tile([E, TPC], F32, tag="u2")
            # u2 = oh2 * (1 - s1)
            nc.vector.scalar_tensor_tensor(
                out=u2, in0=s1, scalar=-1.0, in1=oh2, op0=ALU.mult, op1=ALU.add
            )
            nc.vector.tensor_tensor(out=u2, in0=u2, in1=oh2, op=ALU.mult)
            cnt = rt.tile([E, 2], F32, tag="cnt")
            nc.vector.tensor_reduce(out=cnt[:, 0:1], in_=oh1, axis=AX.X, op=ALU.add)
            nc.vector.tensor_reduce(out=cnt[:, 1:2], in_=oh2, axis=AX.X, op=ALU.add)
            cc_in = dp.tile([E, 2], F32)
            cc_out = dp.tile([E, 2], F32)
            nc.gpsimd.dma_start(cc_in, cnt)
            nc.gpsimd.collective_compute(
                "AllReduce",
                ALU.add,
                replica_groups=[list(range(N_CORES))],
                ins=[cc_in.opt()],
                outs=[cc_out.opt()],
            )
            cntg = rt.tile([E, 2], F32, tag="cntg")
            nc.gpsimd.dma_start(cntg, cc_out)

            # ---- unfused main blocks while the collective runs ------------
            for oc in range(1, FUSE_OC):
                main_oc(oc, False)

            # ---- post-collective combine (vector queue tail) --------------
            alw = rt.tile([E, 2], F32, tag="alw")
            nc.vector.tensor_scalar(
                out=alw, in0=cntg, scalar1=CAPACITY + 0.5, scalar2=None,
                op0=ALU.is_le,
            )
            q2 = rt.tile([E, TPC], F32, tag="q2")
            nc.vector.tensor_scalar(
                out=q2, in0=u2, scalar1=alw[:, 1:2], scalar2=None, op0=ALU.mult
            )
            w_bf = big.tile([E, TPC], BF16)
            nc.vector.scalar_tensor_tensor(
                out=w_bf, in0=u1, scalar=alw[:, 0:1], in1=q2,
                op0=ALU.mult, op1=ALU.add,
            )
            for th in range(TH):
                sl = slice(th * 512, (th + 1) * 512)
                wbr = psc.tile([128, 512], F32, tag="sm", name=f"wbr{th}")
                nc.tensor.matmul(wbr, repp_sb, w_bf[:, sl], start=True, stop=True)
                nc.vector.tensor_tensor(
                    out=tw_bf[:, sl], in0=tmp_sb[:, sl], in1=wbr, op=ALU.mult
                )

            # ---- fused main blocks ----------------------------------------
            for oc in range(FUSE_OC, OC):
                main_oc(oc, True)

            # ---- LoRA tail for the unfused blocks -------------------------
            for oc in range(FUSE_OC):
                for th in range(TH):
                    sl = slice(th * 512, (th + 1) * 512)
                    lp = psc.tile([128, 512], F32, tag="sm", name=f"lp{oc}_{th}")
                    nc.tensor.matmul(
                        lp, b_sb[:, oc * 128 : (oc + 1) * 128], tw_bf[:, sl],
                        start=True, stop=True,
                    )
                    lsb = op_.tile([128, 512], F32, tag="lsb")
                    nc.scalar.activation(lsb, lp, ACT.Copy)
                    nc.sync.dma_start(
                        loraT.ap()[oc * 128 : (oc + 1) * 128, sl], lsb
                    )
    return nc


_CACHE = {}


def _get_nc():
    if "nc" not in _CACHE:
        nc = build_bass()
        nc.finalize()
        _CACHE["nc"] = nc
    return _CACHE["nc"]


def prep_in_maps(inputs):
    x = np.asarray(inputs["x"], dtype=np.float32)
    weight = np.asarray(inputs["weight"], dtype=np.float32)
    xf = x.reshape(N_TOK, IN)
    # wp[oc, p, k*128+c] = weight[oc*128+c, k*128+p]
    wp = np.ascontiguousarray(
        weight.reshape(OC, 128, KT, 128).transpose(0, 3, 2, 1).reshape(OC, 128, KT * 128)
    ).astype(BF)
    a_cat = (
        np.asarray(inputs["lora_A"], np.float32).transpose(1, 0, 2).reshape(IN, ER)
        * ALPHA
    )
    ap_ = np.ascontiguousarray(
        a_cat.reshape(KT, 128, ER).transpose(1, 0, 2).reshape(128, KT * ER)
    ).astype(BF)
    # centered gate weights: LN mean subtraction folded into G1 and gb1
    g1T = np.asarray(inputs["gw1"], np.float32).T  # [IN, 32]
    g1T = g1T - g1T.mean(axis=1, keepdims=True)
    g1p = np.ascontiguousarray(
        g1T.reshape(KT, 128, G4E).transpose(1, 0, 2).reshape(128, KT * G4E)
    ).astype(BF)
    gb1 = np.asarray(inputs["gb1"], np.float32)
    gb1 = gb1 - gb1.mean()
    bp = np.asarray(inputs["lora_B"], np.float32).reshape(ER, OUT).astype(BF)
    g2p = np.ascontiguousarray(np.asarray(inputs["gw2"], np.float32).T).astype(BF)
    repm = np.zeros((E, ER), np.float32)
    for e in range(E):
        repm[e, e * R : (e + 1) * R] = 1.0
    repp = repm.astype(BF)
    gb1c = np.ascontiguousarray(gb1.reshape(G4E, 1))
    gamc = np.ascontiguousarray(
        np.asarray(inputs["ln_gamma"], np.float32).reshape(G4E, 1)
    )
    betc = np.ascontiguousarray(
        np.asarray(inputs["ln_beta"], np.float32).reshape(G4E, 1)
    )
    gb2c = np.ascontiguousarray(np.asarray(inputs["gb2"], np.float32).reshape(E, 1))

    shared = dict(
        wp=wp, ap_=ap_, g1p=g1p, bp=bp, g2p=g2p, repp=repp,
        gb1c=gb1c, gamc=gamc, betc=betc, gb2c=gb2c,
    )
    in_maps = []
    for c in range(N_CORES):
        xs = xf[c * TPC : (c + 1) * TPC]  # [TPC, IN]
        xpc = np.ascontiguousarray(
            xs.T.reshape(KT, 128, TPC).transpose(1, 0, 2).reshape(128, KT * TPC)
        ).astype(BF)
        in_maps.append(dict(xp=xpc, **shared))
    return in_maps


def gather(results):
    out = np.empty((N_TOK, OUT), np.float32)
    for c in range(N_CORES):
        tot = np.array(results[c]["outT"])
        tot[: FUSE_OC * 128] += results[c]["loraT"]
        out[c * TPC : (c + 1) * TPC] = tot.T
    return out.reshape(B, S, OUT)


def kernel(**inputs):
    in_maps = prep_in_maps(inputs)
    nc = _get_nc()
    res = run_bass_kernel_spmd(nc, in_maps, core_ids=list(range(N_CORES)))
    return gather(res.results)


# revision 9
# speedup vs baseline: 1.3315x; 1.0023x over previous
"""LoRA-MoE Linear kernel for Trainium2, 8-core SPMD.

Strategy (token-parallel, bf16 compute):
  - 8192 tokens sharded across 8 cores (1024 each). Host pre-lays-out every
    large operand so each DMA is contiguous per partition.
  - All large matmuls in bf16 (full PE rate); PSUM accumulates fp32.
  - Pass 1 (channel-major): tmp[er,tok] = A^T x and d[32,tok] = (G1-mean)^T x
    share one N=512 moving-x k-loop. The LayerNorm mean subtraction is folded
    into centered gate weights host-side, so PSUM directly yields d.
  - Routing stays expert-major [8,tok]: partition_all_reduce (GpSimd) for
    var / top-2 maxima; zero PE transposes.
  - Engine queues are strict FIFO per engine, so placement = program order:
      PE:     pass1 | oc0 | gates | oc1..oc11 (unfused) | wbr |
              oc12..oc31 (lora fused into the PSUM accumulation) | lora tail
      Vector: pass1 copies, LN, pre-collective routing, post-collective
              combine, tw (nothing after => can't block anything)
      Scalar: Rsqrt, Sigmoid, ALL PSUM->SBUF output copies (ACT.Copy)
      GpSimd: partition reductions, counts-AllReduce + its DMAs
    The counts AllReduce finishes ~160-220us (mesh hop DMAs queue behind
    weight-slab traffic), so LoRA is only fused for oc>=12; oc<12 get a
    separate LoRA pass at the tail written to loraT, host adds.
"""

import numpy as np
import ml_dtypes

import concourse.bacc as bacc
import concourse.bass as bass
import concourse.bass_isa as bass_isa
import concourse.mybir as mybir
import concourse.tile as tile
from concourse.bass_utils import run_bass_kernel_spmd

F32 = mybir.dt.float32
BF16 = mybir.dt.bfloat16
AX = mybir.AxisListType
ALU = mybir.AluOpType
ACT = mybir.ActivationFunctionType
RED = bass_isa.ReduceOp
BF = ml_dtypes.bfloat16

B, S, IN, OUT = 4, 2048, 4096, 4096
E, K, R = 8, 2, 16
CAP_FACTOR = 3.0
ALPHA = 1.0 / R
LN_EPS = 1e-5
N_CORES = 8
N_TOK = B * S               # 8192
TPC = N_TOK // N_CORES      # 1024 tokens per core
G4E = 4 * E                 # 32 gate hidden
ER = E * R                  # 128
KT = IN // 128              # 32 contraction tiles
OC = OUT // 128             # 32 output column blocks
NEG = -1.0e30
CAPACITY = float(int(CAP_FACTOR * N_TOK / E))  # 3072
TH = TPC // 512             # 2 token halves of 512
FUSE_OC = 12                # oc >= FUSE_OC get LoRA fused into main PSUM


def build_bass():
    nc = bacc.Bacc(
        "TRN2", target_bir_lowering=False, debug=False, num_devices=N_CORES
    )
    xp = nc.dram_tensor("xp", [128, KT * TPC], BF16, kind="ExternalInput")
    wp = nc.dram_tensor("wp", [OC, 128, KT * 128], BF16, kind="ExternalInput")
    ap_ = nc.dram_tensor("ap_", [128, KT * ER], BF16, kind="ExternalInput")
    g1p = nc.dram_tensor("g1p", [128, KT * G4E], BF16, kind="ExternalInput")
    bp = nc.dram_tensor("bp", [ER, OUT], BF16, kind="ExternalInput")
    g2p = nc.dram_tensor("g2p", [G4E, E], BF16, kind="ExternalInput")
    repp = nc.dram_tensor("repp", [E, ER], BF16, kind="ExternalInput")
    gb1c = nc.dram_tensor("gb1c", [G4E, 1], F32, kind="ExternalInput")
    gamc = nc.dram_tensor("gamc", [G4E, 1], F32, kind="ExternalInput")
    betc = nc.dram_tensor("betc", [G4E, 1], F32, kind="ExternalInput")
    gb2c = nc.dram_tensor("gb2c", [E, 1], F32, kind="ExternalInput")
    outT = nc.dram_tensor("outT", [OUT, TPC], F32, kind="ExternalOutput")
    loraT = nc.dram_tensor("loraT", [FUSE_OC * 128, TPC], F32, kind="ExternalOutput")

    with tile.TileContext(nc) as tc:
        with (
            tc.tile_pool(name="big", bufs=1) as big,
            tc.tile_pool(name="rt", bufs=1) as rt,
            tc.tile_pool(name="wsl", bufs=3) as wsp,
            tc.tile_pool(name="outp", bufs=4) as op_,
            tc.tile_pool(name="ps_a", bufs=1, space="PSUM") as psa,
            tc.tile_pool(name="ps_c", bufs=2, space="PSUM") as psc,
            tc.tile_pool(name="ps_m", bufs=4, space="PSUM") as psm,
            tc.tile_pool(name="dram", bufs=1, space="DRAM") as dp,
        ):
            # ---- resident loads (x first, fine-grained across queues) -----
            # Single-queue DMA sustains only ~30 GB/s: split x into (k, th)
            # half-tile chunks and a into 4-k chunks so the 16 queues all
            # pull the pass-1 critical path in parallel.
            xT_sb = big.tile([128, KT, TPC], BF16)
            a_sb = big.tile([128, KT, ER], BF16)
            for k in range(KT):
                for th in range(TH):
                    nc.sync.dma_start(
                        xT_sb[:, k, th * 512 : (th + 1) * 512],
                        xp.ap()[:, k * TPC + th * 512 : k * TPC + (th + 1) * 512],
                    )
                if k % 4 == 0:
                    c = k // 4
                    nc.sync.dma_start(
                        a_sb[:, 4 * c : 4 * c + 4],
                        ap_.ap()[:, 4 * c * ER : (4 * c + 4) * ER].rearrange(
                            "p (k e) -> p k e", e=ER
                        ),
                    )
            g1_sb = big.tile([128, KT, G4E], BF16)
            nc.sync.dma_start(
                g1_sb, g1p.ap().rearrange("p (k g) -> p k g", g=G4E)
            )
            b_sb = big.tile([ER, OUT], BF16)
            nc.sync.dma_start(b_sb, bp.ap())
            g2_sb = big.tile([G4E, E], BF16)
            nc.sync.dma_start(g2_sb, g2p.ap())
            repp_sb = big.tile([E, ER], BF16)
            nc.sync.dma_start(repp_sb, repp.ap())
            gb1c_sb = big.tile([G4E, 1], F32)
            nc.sync.dma_start(gb1c_sb, gb1c.ap())
            gamc_sb = big.tile([G4E, 1], F32)
            nc.sync.dma_start(gamc_sb, gamc.ap())
            betc_sb = big.tile([G4E, 1], F32)
            nc.sync.dma_start(betc_sb, betc.ap())
            gb2c_sb = big.tile([E, 1], F32)
            nc.sync.dma_start(gb2c_sb, gb2c.ap())
            eps_sb = big.tile([G4E, 1], F32)
            nc.vector.memset(eps_sb, LN_EPS)

            # ---- pass 1: tmp[er,tok] = A^T x, d[32,tok] = (G1-mean)^T x ---
            tmp_ps = [psa.tile([128, 512], F32, name=f"tmp{t}") for t in range(TH)]
            hT_ps = [
                psc.tile([G4E, 512], F32, tag="sm", name=f"hT{t}") for t in range(TH)
            ]
            for k in range(KT):
                for th in range(TH):
                    nc.tensor.matmul(
                        tmp_ps[th], a_sb[:, k],
                        xT_sb[:, k, th * 512 : (th + 1) * 512],
                        start=(k == 0), stop=(k == KT - 1),
                    )
                for th in range(TH):
                    nc.tensor.matmul(
                        hT_ps[th], g1_sb[:, k],
                        xT_sb[:, k, th * 512 : (th + 1) * 512],
                        start=(k == 0), stop=(k == KT - 1),
                    )
            tmp_sb = big.tile([128, TPC], F32)
            d_sb = big.tile([G4E, TPC], F32)
            for th in range(TH):
                sl = slice(th * 512, (th + 1) * 512)
                nc.vector.tensor_copy(tmp_sb[:, sl], tmp_ps[th])
                # d = (G1-centered)^T x + (gb1 - mean(gb1))  [host-folded]
                nc.vector.tensor_scalar(
                    out=d_sb[:, sl], in0=hT_ps[th], scalar1=gb1c_sb,
                    scalar2=None, op0=ALU.add,
                )

            # ---- main oc0 (hides the LN chain latency) --------------------
            def main_oc(oc, fused):
                wsl = wsp.tile([128, KT, 128], BF16, tag="wsl")
                for q in range(4):
                    nc.sync.dma_start(
                        wsl[:, 8 * q : 8 * q + 8],
                        wp.ap()[oc][:, 8 * q * 128 : (8 * q + 8) * 128].rearrange(
                            "p (k c) -> p k c", c=128
                        ),
                    )
                pos = [
                    psm.tile([128, 512], F32, tag="po", name=f"po{oc}_{t}")
                    for t in range(TH)
                ]
                for k in range(KT):
                    for th in range(TH):
                        nc.tensor.matmul(
                            pos[th], wsl[:, k],
                            xT_sb[:, k, th * 512 : (th + 1) * 512],
                            start=(k == 0),
                            stop=(not fused and k == KT - 1),
                        )
                for th in range(TH):
                    sl = slice(th * 512, (th + 1) * 512)
                    if fused:
                        nc.tensor.matmul(
                            pos[th], b_sb[:, oc * 128 : (oc + 1) * 128],
                            tw_bf[:, sl], start=False, stop=True,
                        )
                    osb = op_.tile([128, 512], F32, tag="osb")
                    nc.scalar.activation(osb, pos[th], ACT.Copy)
                    nc.sync.dma_start(
                        outT.ap()[oc * 128 : (oc + 1) * 128, sl], osb
                    )

            tw_bf = big.tile([128, TPC], BF16)
            main_oc(0, False)
            main_oc(1, False)

            # ---- LayerNorm tail + gate logits -----------------------------
            sq = rt.tile([G4E, TPC], F32, tag="sq")
            nc.vector.tensor_tensor(out=sq, in0=d_sb, in1=d_sb, op=ALU.mult)
            varb = rt.tile([G4E, TPC], F32, tag="varb")
            nc.gpsimd.partition_all_reduce(varb, sq, channels=G4E, reduce_op=RED.add)
            rstd = rt.tile([G4E, TPC], F32, tag="rstd")
            nc.scalar.activation(
                rstd, varb, ACT.Sqrt, bias=eps_sb[:, :], scale=1.0 / G4E
            )
            nc.vector.reciprocal(rstd, rstd)
            nc.vector.tensor_tensor(out=d_sb, in0=d_sb, in1=rstd, op=ALU.mult)
            nc.vector.tensor_scalar(
                out=d_sb, in0=d_sb, scalar1=gamc_sb, scalar2=None, op0=ALU.mult
            )
            nc.vector.tensor_scalar(
                out=d_sb, in0=d_sb, scalar1=betc_sb, scalar2=None, op0=ALU.add
            )
            hn_bf = big.tile([G4E, TPC], BF16)
            nc.vector.tensor_scalar_max(hn_bf, d_sb, 0.0)

            gates = rt.tile([E, TPC], F32, tag="gates")
            for th in range(TH):
                sl = slice(th * 512, (th + 1) * 512)
                g_ps = psc.tile([E, 512], F32, tag="sm", name=f"g{th}")
                nc.tensor.matmul(g_ps, g2_sb, hn_bf[:, sl], start=True, stop=True)
                nc.vector.tensor_scalar(
                    out=gates[:, sl], in0=g_ps, scalar1=gb2c_sb,
                    scalar2=None, op0=ALU.add,
                )

            # ---- top-2 routing, expert-major ------------------------------
            v1 = rt.tile([E, TPC], F32, tag="v1")
            nc.gpsimd.partition_all_reduce(v1, gates, channels=E, reduce_op=RED.max)
            oh1 = rt.tile([E, TPC], F32, tag="oh1")
            nc.vector.tensor_tensor(out=oh1, in0=gates, in1=v1, op=ALU.is_ge)
            msk = rt.tile([E, TPC], F32, tag="msk")
            nc.vector.scalar_tensor_tensor(
                out=msk, in0=oh1, scalar=NEG, in1=gates, op0=ALU.mult, op1=ALU.add
            )
            v2 = rt.tile([E, TPC], F32, tag="v2")
            nc.gpsimd.partition_all_reduce(v2, msk, channels=E, reduce_op=RED.max)
            oh2 = rt.tile([E, TPC], F32, tag="oh2")
            nc.vector.tensor_tensor(out=oh2, in0=msk, in1=v2, op=ALU.is_ge)
            nc.vector.tensor_tensor(out=msk, in0=v1, in1=v2, op=ALU.subtract)
            s1 = rt.tile([E, TPC], F32, tag="s1")
            nc.scalar.activation(s1, msk, ACT.Sigmoid)
            u1 = rt.tile([E, TPC], F32, tag="u1")
            nc.vector.tensor_tensor(out=u1, in0=oh1, in1=s1, op=ALU.mult)
            u2 = rt.tile([E, TPC], F32, tag="u2")
            # u2 = oh2 * (1 - s1)
            nc.vector.scalar_tensor_tensor(
                out=u2, in0=s1, scalar=-1.0, in1=oh2, op0=ALU.mult, op1=ALU.add
            )
            nc.vector.tensor_tensor(out=u2, in0=u2, in1=oh2, op=ALU.mult)
            cnt = rt.tile([E, 2], F32, tag="cnt")
            nc.vector.tensor_reduce(out=cnt[:, 0:1], in_=oh1, axis=AX.X, op=ALU.add)
            nc.vector.tensor_reduce(out=cnt[:, 1:2], in_=oh2, axis=AX.X, op=ALU.add)
            cc_in = dp.tile([E, 2], F32)
            cc_out = dp.tile([E, 2], F32)
            nc.gpsimd.dma_start(cc_in, cnt)
            nc.gpsimd.collective_compute(
                "AllReduce",
                ALU.add,
                replica_groups=[list(range(N_CORES))],
                ins=[cc_in.opt()],
                outs=[cc_out.opt()],
            )
            cntg = rt.tile([E, 2], F32, tag="cntg")
            nc.gpsimd.dma_start(cntg, cc_out)

            # ---- unfused main blocks while the collective runs ------------
            for oc in range(2, FUSE_OC):
                main_oc(oc, False)

            # ---- post-collective combine (vector queue tail) --------------
            alw = rt.tile([E, 2], F32, tag="alw")
            nc.vector.tensor_scalar(
                out=alw, in0=cntg, scalar1=CAPACITY + 0.5, scalar2=None,
                op0=ALU.is_le,
            )
            q2 = rt.tile([E, TPC], F32, tag="q2")
            nc.vector.tensor_scalar(
                out=q2, in0=u2, scalar1=alw[:, 1:2], scalar2=None, op0=ALU.mult
            )
            w_bf = big.tile([E, TPC], BF16)
            nc.vector.scalar_tensor_tensor(
                out=w_bf, in0=u1, scalar=alw[:, 0:1], in1=q2,
                op0=ALU.mult, op1=ALU.add,
            )
            for th in range(TH):
                sl = slice(th * 512, (th + 1) * 512)
                wbr = psc.tile([128, 512], F32, tag="sm", name=f"wbr{th}")
                nc.tensor.matmul(wbr, repp_sb, w_bf[:, sl], start=True, stop=True)
                nc.vector.tensor_tensor(
                    out=tw_bf[:, sl], in0=tmp_sb[:, sl], in1=wbr, op=ALU.mult
                )

            # ---- fused main blocks ----------------------------------------
            for oc in range(FUSE_OC, OC):
                main_oc(oc, True)

            # ---- LoRA tail for the unfused blocks -------------------------
            for oc in range(FUSE_OC):
                for th in range(TH):
                    sl = slice(th * 512, (th + 1) * 512)
                    lp = psc.tile([128, 512], F32, tag="sm", name=f"lp{oc}_{th}")
                    nc.tensor.matmul(
                        lp, b_sb[:, oc * 128 : (oc + 1) * 128], tw_bf[:, sl],
                        start=True, stop=True,
                    )
                    lsb = op_.tile([128, 512], F32, tag="lsb")
                    nc.scalar.activation(lsb, lp, ACT.Copy)
                    nc.sync.dma_start(
                        loraT.ap()[oc * 128 : (oc + 1) * 128, sl], lsb
                    )
    return nc


_CACHE = {}


def _get_nc():
    if "nc" not in _CACHE:
        nc = build_bass()
        nc.finalize()
        _CACHE["nc"] = nc
    return _CACHE["nc"]


def prep_in_maps(inputs):
    x = np.asarray(inputs["x"], dtype=np.float32)
    weight = np.asarray(inputs["weight"], dtype=np.float32)
    xf = x.reshape(N_TOK, IN)
    # wp[oc, p, k*128+c] = weight[oc*128+c, k*128+p]
    wp = np.ascontiguousarray(
        weight.reshape(OC, 128, KT, 128).transpose(0, 3, 2, 1).reshape(OC, 128, KT * 128)
    ).astype(BF)
    a_cat = (
        np.asarray(inputs["lora_A"], np.float32).transpose(1, 0, 2).reshape(IN, ER)
        * ALPHA
    )
    ap_ = np.ascontiguousarray(
        a_cat.reshape(KT, 128, ER).transpose(1, 0, 2).reshape(128, KT * ER)
    ).astype(BF)
    # centered gate weights: LN mean subtraction folded into G1 and gb1
    g1T = np.asarray(inputs["gw1"], np.float32).T  # [IN, 32]
    g1T = g1T - g1T.mean(axis=1, keepdims=True)
    g1p = np.ascontiguousarray(
        g1T.reshape(KT, 128, G4E).transpose(1, 0, 2).reshape(128, KT * G4E)
    ).astype(BF)
    gb1 = np.asarray(inputs["gb1"], np.float32)
    gb1 = gb1 - gb1.mean()
    bp = np.asarray(inputs["lora_B"], np.float32).reshape(ER, OUT).astype(BF)
    g2p = np.ascontiguousarray(np.asarray(inputs["gw2"], np.float32).T).astype(BF)
    repm = np.zeros((E, ER), np.float32)
    for e in range(E):
        repm[e, e * R : (e + 1) * R] = 1.0
    repp = repm.astype(BF)
    gb1c = np.ascontiguousarray(gb1.reshape(G4E, 1))
    gamc = np.ascontiguousarray(
        np.asarray(inputs["ln_gamma"], np.float32).reshape(G4E, 1)
    )
    betc = np.ascontiguousarray(
        np.asarray(inputs["ln_beta"], np.float32).reshape(G4E, 1)
    )
    gb2c = np.ascontiguousarray(np.asarray(inputs["gb2"], np.float32).reshape(E, 1))

    shared = dict(
        wp=wp, ap_=ap_, g1p=g1p, bp=bp, g2p=g2p, repp=repp,
        gb1c=gb1c, gamc=gamc, betc=betc, gb2c=gb2c,
    )
    in_maps = []
    for c in range(N_CORES):
        xs = xf[c * TPC : (c + 1) * TPC]  # [TPC, IN]
        xpc = np.ascontiguousarray(
            xs.T.reshape(KT, 128, TPC).transpose(1, 0, 2).reshape(128, KT * TPC)
        ).astype(BF)
        in_maps.append(dict(xp=xpc, **shared))
    return in_maps


def gather(results):
    out = np.empty((N_TOK, OUT), np.float32)
    for c in range(N_CORES):
        tot = np.array(results[c]["outT"])
        tot[: FUSE_OC * 128] += results[c]["loraT"]
        out[c * TPC : (c + 1) * TPC] = tot.T
    return out.reshape(B, S, OUT)


def kernel(**inputs):
    in_maps = prep_in_maps(inputs)
    nc = _get_nc()
    res = run_bass_kernel_spmd(nc, in_maps, core_ids=list(range(N_CORES)))
    return gather(res.results)


# revision 12
# speedup vs baseline: 1.3362x; 1.0035x over previous
"""LoRA-MoE Linear kernel for Trainium2, 8-core SPMD.

Strategy (token-parallel, bf16 compute):
  - 8192 tokens sharded across 8 cores (1024 each). Host pre-lays-out every
    large operand so each DMA is contiguous per partition.
  - All large matmuls in bf16 (full PE rate); PSUM accumulates fp32.
  - Pass 1 (channel-major): tmp[er,tok] = A^T x and d[32,tok] = (G1-mean)^T x
    share one N=512 moving-x k-loop. The LayerNorm mean subtraction is folded
    into centered gate weights host-side, so PSUM directly yields d.
  - Routing stays expert-major [8,tok]: partition_all_reduce (GpSimd) for
    var / top-2 maxima; zero PE transposes.
  - Engine queues are strict FIFO per engine, so placement = program order:
      PE:     pass1 | oc0 | gates | oc1..oc11 (unfused) | wbr |
              oc12..oc31 (lora fused into the PSUM accumulation) | lora tail
      Vector: pass1 copies, LN, pre-collective routing, post-collective
              combine, tw (nothing after => can't block anything)
      Scalar: Rsqrt, Sigmoid, ALL PSUM->SBUF output copies (ACT.Copy)
      GpSimd: partition reductions, counts-AllReduce + its DMAs
    The counts AllReduce finishes ~160-220us (mesh hop DMAs queue behind
    weight-slab traffic), so LoRA is only fused for oc>=12; oc<12 get a
    separate LoRA pass at the tail written to loraT, host adds.
"""

import numpy as np
import ml_dtypes

import concourse.bacc as bacc
import concourse.bass as bass
import concourse.bass_isa as bass_isa
import concourse.mybir as mybir
import concourse.tile as tile
from concourse.bass_utils import run_bass_kernel_spmd

F32 = mybir.dt.float32
BF16 = mybir.dt.bfloat16
AX = mybir.AxisListType
ALU = mybir.AluOpType
ACT = mybir.ActivationFunctionType
RED = bass_isa.ReduceOp
BF = ml_dtypes.bfloat16

B, S, IN, OUT = 4, 2048, 4096, 4096
E, K, R = 8, 2, 16
CAP_FACTOR = 3.0
ALPHA = 1.0 / R
LN_EPS = 1e-5
N_CORES = 8
N_TOK = B * S               # 8192
TPC = N_TOK // N_CORES      # 1024 tokens per core
G4E = 4 * E                 # 32 gate hidden
ER = E * R                  # 128
KT = IN // 128              # 32 contraction tiles
OC = OUT // 128             # 32 output column blocks
NEG = -1.0e30
CAPACITY = float(int(CAP_FACTOR * N_TOK / E))  # 3072
TH = TPC // 512             # 2 token halves of 512
FUSE_OC = 12                # oc >= FUSE_OC get LoRA fused into main PSUM


def build_bass():
    nc = bacc.Bacc(
        "TRN2", target_bir_lowering=False, debug=False, num_devices=N_CORES
    )
    xp = nc.dram_tensor("xp", [128, KT * TPC], BF16, kind="ExternalInput")
    wp = nc.dram_tensor("wp", [OC, 128, KT * 128], BF16, kind="ExternalInput")
    ap_ = nc.dram_tensor("ap_", [128, KT * ER], BF16, kind="ExternalInput")
    g1p = nc.dram_tensor("g1p", [128, KT * G4E], BF16, kind="ExternalInput")
    bp = nc.dram_tensor("bp", [ER, OUT], BF16, kind="ExternalInput")
    g2p = nc.dram_tensor("g2p", [G4E, E], BF16, kind="ExternalInput")
    repp = nc.dram_tensor("repp", [E, ER], BF16, kind="ExternalInput")
    gb1c = nc.dram_tensor("gb1c", [G4E, 1], F32, kind="ExternalInput")
    gamc = nc.dram_tensor("gamc", [G4E, 1], F32, kind="ExternalInput")
    betc = nc.dram_tensor("betc", [G4E, 1], F32, kind="ExternalInput")
    gb2c = nc.dram_tensor("gb2c", [E, 1], F32, kind="ExternalInput")
    outT = nc.dram_tensor("outT", [OUT, TPC], F32, kind="ExternalOutput")
    loraT = nc.dram_tensor("loraT", [FUSE_OC * 128, TPC], F32, kind="ExternalOutput")

    with tile.TileContext(nc) as tc:
        with (
            tc.tile_pool(name="big", bufs=1) as big,
            tc.tile_pool(name="rt", bufs=1) as rt,
            tc.tile_pool(name="wsl", bufs=3) as wsp,
            tc.tile_pool(name="outp", bufs=4) as op_,
            tc.tile_pool(name="ps_a", bufs=1, space="PSUM") as psa,
            tc.tile_pool(name="ps_c", bufs=2, space="PSUM") as psc,
            tc.tile_pool(name="ps_m", bufs=4, space="PSUM") as psm,
            tc.tile_pool(name="dram", bufs=1, space="DRAM") as dp,
        ):
            # ---- resident loads (x first, fine-grained across queues) -----
            # Single-queue DMA sustains only ~30 GB/s: split x into (k, th)
            # half-tile chunks and a into 4-k chunks so the 16 queues all
            # pull the pass-1 critical path in parallel.
            xT_sb = big.tile([128, KT, TPC], BF16)
            a_sb = big.tile([128, KT, ER], BF16)
            for k in range(KT):
                for th in range(TH):
                    nc.sync.dma_start(
                        xT_sb[:, k, th * 512 : (th + 1) * 512],
                        xp.ap()[:, k * TPC + th * 512 : k * TPC + (th + 1) * 512],
                    )
                if k % 4 == 0:
                    c = k // 4
                    nc.sync.dma_start(
                        a_sb[:, 4 * c : 4 * c + 4],
                        ap_.ap()[:, 4 * c * ER : (4 * c + 4) * ER].rearrange(
                            "p (k e) -> p k e", e=ER
                        ),
                    )
            g1_sb = big.tile([128, KT, G4E], BF16)
            nc.sync.dma_start(
                g1_sb, g1p.ap().rearrange("p (k g) -> p k g", g=G4E)
            )
            b_sb = big.tile([ER, OUT], BF16)
            nc.sync.dma_start(b_sb, bp.ap())
            g2_sb = big.tile([G4E, E], BF16)
            nc.sync.dma_start(g2_sb, g2p.ap())
            repp_sb = big.tile([E, ER], BF16)
            nc.sync.dma_start(repp_sb, repp.ap())
            gb1c_sb = big.tile([G4E, 1], F32)
            nc.sync.dma_start(gb1c_sb, gb1c.ap())
            gamc_sb = big.tile([G4E, 1], F32)
            nc.sync.dma_start(gamc_sb, gamc.ap())
            betc_sb = big.tile([G4E, 1], F32)
            nc.sync.dma_start(betc_sb, betc.ap())
            gb2c_sb = big.tile([E, 1], F32)
            nc.sync.dma_start(gb2c_sb, gb2c.ap())
            eps_sb = big.tile([G4E, 1], F32)
            nc.vector.memset(eps_sb, LN_EPS)

            # ---- pass 1 + oc0/oc1 main blocks, one x-chasing k-loop -------
            # 8 MMs per k-tile (~2.2us) vs ~0.7us DMA arrival: PE saturates
            # from the second tile and the whole x load hides under compute.
            tmp_ps = [psa.tile([128, 512], F32, name=f"tmp{t}") for t in range(TH)]
            hT_ps = [
                psc.tile([G4E, 512], F32, tag="sm", name=f"hT{t}") for t in range(TH)
            ]
            wsl01 = []
            pos01 = []
            for oc in range(2):
                wsl = wsp.tile([128, KT, 128], BF16, tag="wsl")
                for q in range(4):
                    nc.sync.dma_start(
                        wsl[:, 8 * q : 8 * q + 8],
                        wp.ap()[oc][:, 8 * q * 128 : (8 * q + 8) * 128].rearrange(
                            "p (k c) -> p k c", c=128
                        ),
                    )
                wsl01.append(wsl)
                pos01.append(
                    [
                        psm.tile([128, 512], F32, tag="po", name=f"po{oc}_{t}")
                        for t in range(TH)
                    ]
                )
            for k in range(KT):
                first, last = k == 0, k == KT - 1
                for th in range(TH):
                    nc.tensor.matmul(
                        tmp_ps[th], a_sb[:, k],
                        xT_sb[:, k, th * 512 : (th + 1) * 512],
                        start=first, stop=last,
                    )
                for th in range(TH):
                    nc.tensor.matmul(
                        hT_ps[th], g1_sb[:, k],
                        xT_sb[:, k, th * 512 : (th + 1) * 512],
                        start=first, stop=last,
                    )
                for oc in range(2):
                    for th in range(TH):
                        nc.tensor.matmul(
                            pos01[oc][th], wsl01[oc][:, k],
                            xT_sb[:, k, th * 512 : (th + 1) * 512],
                            start=first, stop=last,
                        )
            tmp_sb = big.tile([128, TPC], F32)
            d_sb = big.tile([G4E, TPC], F32)
            for th in range(TH):
                sl = slice(th * 512, (th + 1) * 512)
                nc.vector.tensor_copy(tmp_sb[:, sl], tmp_ps[th])
                # d = (G1-centered)^T x + (gb1 - mean(gb1))  [host-folded]
                nc.vector.tensor_scalar(
                    out=d_sb[:, sl], in0=hT_ps[th], scalar1=gb1c_sb,
                    scalar2=None, op0=ALU.add,
                )
            for oc in range(2):
                for th in range(TH):
                    sl = slice(th * 512, (th + 1) * 512)
                    osb = op_.tile([128, 512], F32, tag="osb")
                    nc.scalar.activation(osb, pos01[oc][th], ACT.Copy)
                    nc.sync.dma_start(
                        outT.ap()[oc * 128 : (oc + 1) * 128, sl], osb
                    )

            def main_oc(oc, fused):
                wsl = wsp.tile([128, KT, 128], BF16, tag="wsl")
                nc.sync.dma_start(
                    wsl, wp.ap()[oc].rearrange("p (k c) -> p k c", c=128)
                )
                pos = [
                    psm.tile([128, 512], F32, tag="po", name=f"po{oc}_{t}")
                    for t in range(TH)
                ]
                for k in range(KT):
                    for th in range(TH):
                        nc.tensor.matmul(
                            pos[th], wsl[:, k],
                            xT_sb[:, k, th * 512 : (th + 1) * 512],
                            start=(k == 0),
                            stop=(not fused and k == KT - 1),
                        )
                for th in range(TH):
                    sl = slice(th * 512, (th + 1) * 512)
                    if fused:
                        nc.tensor.matmul(
                            pos[th], b_sb[:, oc * 128 : (oc + 1) * 128],
                            tw_bf[:, sl], start=False, stop=True,
                        )
                    osb = op_.tile([128, 512], F32, tag="osb")
                    nc.scalar.activation(osb, pos[th], ACT.Copy)
                    nc.sync.dma_start(
                        outT.ap()[oc * 128 : (oc + 1) * 128, sl], osb
                    )

            tw_bf = big.tile([128, TPC], BF16)
            main_oc(2, False)
            main_oc(3, False)

            # ---- LayerNorm tail + gate logits -----------------------------
            sq = rt.tile([G4E, TPC], F32, tag="sq")
            nc.vector.tensor_tensor(out=sq, in0=d_sb, in1=d_sb, op=ALU.mult)
            varb = rt.tile([G4E, TPC], F32, tag="varb")
            nc.gpsimd.partition_all_reduce(varb, sq, channels=G4E, reduce_op=RED.add)
            rstd = rt.tile([G4E, TPC], F32, tag="rstd")
            nc.scalar.activation(
                rstd, varb, ACT.Sqrt, bias=eps_sb[:, :], scale=1.0 / G4E
            )
            nc.vector.reciprocal(rstd, rstd)
            nc.vector.tensor_tensor(out=d_sb, in0=d_sb, in1=rstd, op=ALU.mult)
            nc.vector.tensor_scalar(
                out=d_sb, in0=d_sb, scalar1=gamc_sb, scalar2=None, op0=ALU.mult
            )
            nc.vector.tensor_scalar(
                out=d_sb, in0=d_sb, scalar1=betc_sb, scalar2=None, op0=ALU.add
            )
            hn_bf = big.tile([G4E, TPC], BF16)
            nc.vector.tensor_scalar_max(hn_bf, d_sb, 0.0)

            gates = rt.tile([E, TPC], F32, tag="gates")
            for th in range(TH):
                sl = slice(th * 512, (th + 1) * 512)
                g_ps = psc.tile([E, 512], F32, tag="sm", name=f"g{th}")
                nc.tensor.matmul(g_ps, g2_sb, hn_bf[:, sl], start=True, stop=True)
                nc.vector.tensor_scalar(
                    out=gates[:, sl], in0=g_ps, scalar1=gb2c_sb,
                    scalar2=None, op0=ALU.add,
                )

            # ---- top-2 routing, expert-major ------------------------------
            v1 = rt.tile([E, TPC], F32, tag="v1")
            nc.gpsimd.partition_all_reduce(v1, gates, channels=E, reduce_op=RED.max)
            oh1 = rt.tile([E, TPC], F32, tag="oh1")
            nc.vector.tensor_tensor(out=oh1, in0=gates, in1=v1, op=ALU.is_ge)
            msk = rt.tile([E, TPC], F32, tag="msk")
            nc.vector.scalar_tensor_tensor(
                out=msk, in0=oh1, scalar=NEG, in1=gates, op0=ALU.mult, op1=ALU.add
            )
            v2 = rt.tile([E, TPC], F32, tag="v2")
            nc.gpsimd.partition_all_reduce(v2, msk, channels=E, reduce_op=RED.max)
            oh2 = rt.tile([E, TPC], F32, tag="oh2")
            nc.vector.tensor_tensor(out=oh2, in0=msk, in1=v2, op=ALU.is_ge)
            nc.vector.tensor_tensor(out=msk, in0=v1, in1=v2, op=ALU.subtract)
            s1 = rt.tile([E, TPC], F32, tag="s1")
            nc.scalar.activation(s1, msk, ACT.Sigmoid)
            u1 = rt.tile([E, TPC], F32, tag="u1")
            nc.vector.tensor_tensor(out=u1, in0=oh1, in1=s1, op=ALU.mult)
            u2 = rt.tile([E, TPC], F32, tag="u2")
            # u2 = oh2 * (1 - s1)
            nc.vector.scalar_tensor_tensor(
                out=u2, in0=s1, scalar=-1.0, in1=oh2, op0=ALU.mult, op1=ALU.add
            )
            nc.vector.tensor_tensor(out=u2, in0=u2, in1=oh2, op=ALU.mult)
            cnt = rt.tile([E, 2], F32, tag="cnt")
            nc.vector.tensor_reduce(out=cnt[:, 0:1], in_=oh1, axis=AX.X, op=ALU.add)
            nc.vector.tensor_reduce(out=cnt[:, 1:2], in_=oh2, axis=AX.X, op=ALU.add)
            cc_in = dp.tile([E, 2], F32)
            cc_out = dp.tile([E, 2], F32)
            nc.gpsimd.dma_start(cc_in, cnt)
            nc.gpsimd.collective_compute(
                "AllReduce",
                ALU.add,
                replica_groups=[list(range(N_CORES))],
                ins=[cc_in.opt()],
                outs=[cc_out.opt()],
            )
            cntg = rt.tile([E, 2], F32, tag="cntg")
            nc.gpsimd.dma_start(cntg, cc_out)

            # ---- unfused main blocks while the collective runs ------------
            for oc in range(4, FUSE_OC):
                main_oc(oc, False)

            # ---- post-collective combine (vector queue tail) --------------
            alw = rt.tile([E, 2], F32, tag="alw")
            nc.vector.tensor_scalar(
                out=alw, in0=cntg, scalar1=CAPACITY + 0.5, scalar2=None,
                op0=ALU.is_le,
            )
            q2 = rt.tile([E, TPC], F32, tag="q2")
            nc.vector.tensor_scalar(
                out=q2, in0=u2, scalar1=alw[:, 1:2], scalar2=None, op0=ALU.mult
            )
            w_bf = big.tile([E, TPC], BF16)
            nc.vector.scalar_tensor_tensor(
                out=w_bf, in0=u1, scalar=alw[:, 0:1], in1=q2,
                op0=ALU.mult, op1=ALU.add,
            )
            for th in range(TH):
                sl = slice(th * 512, (th + 1) * 512)
                wbr = psc.tile([128, 512], F32, tag="sm", name=f"wbr{th}")
                nc.tensor.matmul(wbr, repp_sb, w_bf[:, sl], start=True, stop=True)
                nc.vector.tensor_tensor(
                    out=tw_bf[:, sl], in0=tmp_sb[:, sl], in1=wbr, op=ALU.mult
                )

            # ---- fused main blocks ----------------------------------------
            for oc in range(FUSE_OC, OC):
                main_oc(oc, True)

            # ---- LoRA tail for the unfused blocks -------------------------
            for oc in range(FUSE_OC):
                for th in range(TH):
                    sl = slice(th * 512, (th + 1) * 512)
                    lp = psm.tile([128, 512], F32, tag="po", name=f"lp{oc}_{th}")
                    nc.tensor.matmul(
                        lp, b_sb[:, oc * 128 : (oc + 1) * 128], tw_bf[:, sl],
                        start=True, stop=True,
                    )
                    lsb = op_.tile([128, 512], F32, tag="lsb")
                    if th == 0:
                        nc.scalar.activation(lsb, lp, ACT.Copy)
                    else:
                        nc.vector.tensor_copy(lsb, lp)
                    nc.sync.dma_start(
                        loraT.ap()[oc * 128 : (oc + 1) * 128, sl], lsb
                    )
    return nc


_CACHE = {}


def _get_nc():
    if "nc" not in _CACHE:
        nc = build_bass()
        nc.finalize()
        _CACHE["nc"] = nc
    return _CACHE["nc"]


def prep_in_maps(inputs):
    x = np.asarray(inputs["x"], dtype=np.float32)
    weight = np.asarray(inputs["weight"], dtype=np.float32)
    xf = x.reshape(N_TOK, IN)
    # wp[oc, p, k*128+c] = weight[oc*128+c, k*128+p]
    wp = np.ascontiguousarray(
        weight.reshape(OC, 128, KT, 128).transpose(0, 3, 2, 1).reshape(OC, 128, KT * 128)
    ).astype(BF)
    a_cat = (
        np.asarray(inputs["lora_A"], np.float32).transpose(1, 0, 2).reshape(IN, ER)
        * ALPHA
    )
    ap_ = np.ascontiguousarray(
        a_cat.reshape(KT, 128, ER).transpose(1, 0, 2).reshape(128, KT * ER)
    ).astype(BF)
    # centered gate weights: LN mean subtraction folded into G1 and gb1
    g1T = np.asarray(inputs["gw1"], np.float32).T  # [IN, 32]
    g1T = g1T - g1T.mean(axis=1, keepdims=True)
    g1p = np.ascontiguousarray(
        g1T.reshape(KT, 128, G4E).transpose(1, 0, 2).reshape(128, KT * G4E)
    ).astype(BF)
    gb1 = np.asarray(inputs["gb1"], np.float32)
    gb1 = gb1 - gb1.mean()
    bp = np.asarray(inputs["lora_B"], np.float32).reshape(ER, OUT).astype(BF)
    g2p = np.ascontiguousarray(np.asarray(inputs["gw2"], np.float32).T).astype(BF)
    repm = np.zeros((E, ER), np.float32)
    for e in range(E):
        repm[e, e * R : (e + 1) * R] = 1.0
    repp = repm.astype(BF)
    gb1c = np.ascontiguousarray(gb1.reshape(G4E, 1))
    gamc = np.ascontiguousarray(
        np.asarray(inputs["ln_gamma"], np.float32).reshape(G4E, 1)
    )
    betc = np.ascontiguousarray(
        np.asarray(inputs["ln_beta"], np.float32).reshape(G4E, 1)
    )
    gb2c = np.ascontiguousarray(np.asarray(inputs["gb2"], np.float32).reshape(E, 1))

    shared = dict(
        wp=wp, ap_=ap_, g1p=g1p, bp=bp, g2p=g2p, repp=repp,
        gb1c=gb1c, gamc=gamc, betc=betc, gb2c=gb2c,
    )
    in_maps = []
    for c in range(N_CORES):
        xs = xf[c * TPC : (c + 1) * TPC]  # [TPC, IN]
        xpc = np.ascontiguousarray(
            xs.T.reshape(KT, 128, TPC).transpose(1, 0, 2).reshape(128, KT * TPC)
        ).astype(BF)
        in_maps.append(dict(xp=xpc, **shared))
    return in_maps


def gather(results):
    out = np.empty((N_TOK, OUT), np.float32)
    for c in range(N_CORES):
        tot = np.array(results[c]["outT"])
        tot[: FUSE_OC * 128] += results[c]["loraT"]
        out[c * TPC : (c + 1) * TPC] = tot.T
    return out.reshape(B, S, OUT)


def kernel(**inputs):
    in_maps = prep_in_maps(inputs)
    nc = _get_nc()
    res = run_bass_kernel_spmd(nc, in_maps, core_ids=list(range(N_CORES)))
    return gather(res.results)


# revision 13
# speedup vs baseline: 1.3665x; 1.0227x over previous
"""LoRA-MoE Linear kernel for Trainium2, 8-core SPMD.

Strategy (token-parallel, bf16 compute):
  - 8192 tokens sharded across 8 cores (1024 each). Host pre-lays-out every
    large operand so each DMA is contiguous per partition.
  - All large matmuls in bf16 (full PE rate); PSUM accumulates fp32.
  - Pass 1 (channel-major): tmp[er,tok] = A^T x and d[32,tok] = (G1-mean)^T x
    share one N=512 moving-x k-loop. The LayerNorm mean subtraction is folded
    into centered gate weights host-side, so PSUM directly yields d.
  - Routing stays expert-major [8,tok]: partition_all_reduce (GpSimd) for
    var / top-2 maxima; zero PE transposes.
  - Engine queues are strict FIFO per engine, so placement = program order:
      PE:     pass1 | oc0 | gates | oc1..oc11 (unfused) | wbr |
              oc12..oc31 (lora fused into the PSUM accumulation) | lora tail
      Vector: pass1 copies, LN, pre-collective routing, post-collective
              combine, tw (nothing after => can't block anything)
      Scalar: Rsqrt, Sigmoid, ALL PSUM->SBUF output copies (ACT.Copy)
      GpSimd: partition reductions, counts-AllReduce + its DMAs
    The counts AllReduce finishes ~160-220us (mesh hop DMAs queue behind
    weight-slab traffic), so LoRA is only fused for oc>=12; oc<12 get a
    separate LoRA pass at the tail written to loraT, host adds.
"""

import numpy as np
import ml_dtypes

import concourse.bacc as bacc
import concourse.bass as bass
import concourse.bass_isa as bass_isa
import concourse.mybir as mybir
import concourse.tile as tile
from concourse.bass_utils import run_bass_kernel_spmd

F32 = mybir.dt.float32
BF16 = mybir.dt.bfloat16
AX = mybir.AxisListType
ALU = mybir.AluOpType
ACT = mybir.ActivationFunctionType
RED = bass_isa.ReduceOp
BF = ml_dtypes.bfloat16

B, S, IN, OUT = 4, 2048, 4096, 4096
E, K, R = 8, 2, 16
CAP_FACTOR = 3.0
ALPHA = 1.0 / R
LN_EPS = 1e-5
N_CORES = 8
N_TOK = B * S               # 8192
TPC = N_TOK // N_CORES      # 1024 tokens per core
G4E = 4 * E                 # 32 gate hidden
ER = E * R                  # 128
KT = IN // 128              # 32 contraction tiles
OC = OUT // 128             # 32 output column blocks
NEG = -1.0e30
CAPACITY = float(int(CAP_FACTOR * N_TOK / E))  # 3072
TH = TPC // 512             # 2 token halves of 512
FUSE_OC = 12                # oc >= FUSE_OC get LoRA fused into main PSUM


def build_bass():
    nc = bacc.Bacc(
        "TRN2", target_bir_lowering=False, debug=False, num_devices=N_CORES
    )
    xp = nc.dram_tensor("xp", [128, KT * TPC], BF16, kind="ExternalInput")
    wp = nc.dram_tensor("wp", [OC, 128, KT * 128], BF16, kind="ExternalInput")
    ap_ = nc.dram_tensor("ap_", [128, KT * ER], BF16, kind="ExternalInput")
    g1p = nc.dram_tensor("g1p", [128, KT * G4E], BF16, kind="ExternalInput")
    bp = nc.dram_tensor("bp", [ER, OUT], BF16, kind="ExternalInput")
    g2p = nc.dram_tensor("g2p", [G4E, E], BF16, kind="ExternalInput")
    repp = nc.dram_tensor("repp", [E, ER], BF16, kind="ExternalInput")
    gb1c = nc.dram_tensor("gb1c", [G4E, 1], F32, kind="ExternalInput")
    gamc = nc.dram_tensor("gamc", [G4E, 1], F32, kind="ExternalInput")
    betc = nc.dram_tensor("betc", [G4E, 1], F32, kind="ExternalInput")
    gb2c = nc.dram_tensor("gb2c", [E, 1], F32, kind="ExternalInput")
    outT = nc.dram_tensor("outT", [OUT, TPC], F32, kind="ExternalOutput")
    loraT = nc.dram_tensor("loraT", [FUSE_OC * 128, TPC], F32, kind="ExternalOutput")

    with tile.TileContext(nc) as tc:
        with (
            tc.tile_pool(name="big", bufs=1) as big,
            tc.tile_pool(name="rt", bufs=1) as rt,
            tc.tile_pool(name="wsl", bufs=3) as wsp,
            tc.tile_pool(name="outp", bufs=4) as op_,
            tc.tile_pool(name="ps_a", bufs=1, space="PSUM") as psa,
            tc.tile_pool(name="ps_c", bufs=2, space="PSUM") as psc,
            tc.tile_pool(name="ps_m", bufs=4, space="PSUM") as psm,
            tc.tile_pool(name="dram", bufs=1, space="DRAM") as dp,
        ):
            # ---- resident loads (x first, fine-grained across queues) -----
            # Single-queue DMA sustains only ~30 GB/s: split x into (k, th)
            # half-tile chunks and a into 4-k chunks so the 16 queues all
            # pull the pass-1 critical path in parallel.
            xT_sb = big.tile([128, KT, TPC], BF16)
            a_sb = big.tile([128, KT, ER], BF16)
            for k in range(KT):
                nc.sync.dma_start(
                    xT_sb[:, k], xp.ap()[:, k * TPC : (k + 1) * TPC]
                )
                if k % 8 == 0:
                    c = k // 8
                    nc.sync.dma_start(
                        a_sb[:, 8 * c : 8 * c + 8],
                        ap_.ap()[:, 8 * c * ER : (8 * c + 8) * ER].rearrange(
                            "p (k e) -> p k e", e=ER
                        ),
                    )
            g1_sb = big.tile([128, KT, G4E], BF16)
            nc.sync.dma_start(
                g1_sb, g1p.ap().rearrange("p (k g) -> p k g", g=G4E)
            )
            b_sb = big.tile([ER, OUT], BF16)
            for hh in range(2):
                nc.sync.dma_start(
                    b_sb[:, hh * 2048 : (hh + 1) * 2048],
                    bp.ap()[:, hh * 2048 : (hh + 1) * 2048],
                )
            g2_sb = big.tile([G4E, E], BF16)
            nc.sync.dma_start(g2_sb, g2p.ap())
            repp_sb = big.tile([E, ER], BF16)
            nc.sync.dma_start(repp_sb, repp.ap())
            gb1c_sb = big.tile([G4E, 1], F32)
            nc.sync.dma_start(gb1c_sb, gb1c.ap())
            gamc_sb = big.tile([G4E, 1], F32)
            nc.sync.dma_start(gamc_sb, gamc.ap())
            betc_sb = big.tile([G4E, 1], F32)
            nc.sync.dma_start(betc_sb, betc.ap())
            gb2c_sb = big.tile([E, 1], F32)
            nc.sync.dma_start(gb2c_sb, gb2c.ap())
            eps_sb = big.tile([G4E, 1], F32)
            nc.vector.memset(eps_sb, LN_EPS)

            # ---- pass 1 + oc0/oc1 main blocks, one x-chasing k-loop -------
            # 8 MMs per k-tile (~2.2us) vs ~0.7us DMA arrival: PE saturates
            # from the second tile and the whole x load hides under compute.
            tmp_ps = [psa.tile([128, 512], F32, name=f"tmp{t}") for t in range(TH)]
            hT_ps = [
                psc.tile([G4E, 512], F32, tag="sm", name=f"hT{t}") for t in range(TH)
            ]
            wsl01 = []
            pos01 = []
            for oc in range(2):
                wsl = wsp.tile([128, KT, 128], BF16, tag="wsl")
                for q in range(4):
                    nc.sync.dma_start(
                        wsl[:, 8 * q : 8 * q + 8],
                        wp.ap()[oc][:, 8 * q * 128 : (8 * q + 8) * 128].rearrange(
                            "p (k c) -> p k c", c=128
                        ),
                    )
                wsl01.append(wsl)
                pos01.append(
                    [
                        psm.tile([128, 512], F32, tag="po", name=f"po{oc}_{t}")
                        for t in range(TH)
                    ]
                )
            for k in range(KT):
                first, last = k == 0, k == KT - 1
                for th in range(TH):
                    nc.tensor.matmul(
                        tmp_ps[th], a_sb[:, k],
                        xT_sb[:, k, th * 512 : (th + 1) * 512],
                        start=first, stop=last,
                    )
                for th in range(TH):
                    nc.tensor.matmul(
                        hT_ps[th], g1_sb[:, k],
                        xT_sb[:, k, th * 512 : (th + 1) * 512],
                        start=first, stop=last,
                    )
                for oc in range(2):
                    for th in range(TH):
                        nc.tensor.matmul(
                            pos01[oc][th], wsl01[oc][:, k],
                            xT_sb[:, k, th * 512 : (th + 1) * 512],
                            start=first, stop=last,
                        )
            tmp_sb = big.tile([128, TPC], F32)
            d_sb = big.tile([G4E, TPC], F32)
            for th in range(TH):
                sl = slice(th * 512, (th + 1) * 512)
                nc.vector.tensor_copy(tmp_sb[:, sl], tmp_ps[th])
                # d = (G1-centered)^T x + (gb1 - mean(gb1))  [host-folded]
                nc.vector.tensor_scalar(
                    out=d_sb[:, sl], in0=hT_ps[th], scalar1=gb1c_sb,
                    scalar2=None, op0=ALU.add,
                )
            for oc in range(2):
                for th in range(TH):
                    sl = slice(th * 512, (th + 1) * 512)
                    osb = op_.tile([128, 512], F32, tag="osb")
                    nc.scalar.activation(osb, pos01[oc][th], ACT.Copy)
                    nc.sync.dma_start(
                        outT.ap()[oc * 128 : (oc + 1) * 128, sl], osb
                    )

            def main_oc(oc, fused):
                wsl = wsp.tile([128, KT, 128], BF16, tag="wsl")
                nc.sync.dma_start(
                    wsl, wp.ap()[oc].rearrange("p (k c) -> p k c", c=128)
                )
                pos = [
                    psm.tile([128, 512], F32, tag="po", name=f"po{oc}_{t}")
                    for t in range(TH)
                ]
                for k in range(KT):
                    for th in range(TH):
                        nc.tensor.matmul(
                            pos[th], wsl[:, k],
                            xT_sb[:, k, th * 512 : (th + 1) * 512],
                            start=(k == 0),
                            stop=(not fused and k == KT - 1),
                        )
                for th in range(TH):
                    sl = slice(th * 512, (th + 1) * 512)
                    if fused:
                        nc.tensor.matmul(
                            pos[th], b_sb[:, oc * 128 : (oc + 1) * 128],
                            tw_bf[:, sl], start=False, stop=True,
                        )
                    osb = op_.tile([128, 512], F32, tag="osb")
                    nc.scalar.activation(osb, pos[th], ACT.Copy)
                    nc.sync.dma_start(
                        outT.ap()[oc * 128 : (oc + 1) * 128, sl], osb
                    )

            tw_bf = big.tile([128, TPC], BF16)
            main_oc(2, False)
            main_oc(3, False)

            # ---- LayerNorm tail + gate logits -----------------------------
            sq = rt.tile([G4E, TPC], F32, tag="sq")
            nc.vector.tensor_tensor(out=sq, in0=d_sb, in1=d_sb, op=ALU.mult)
            varb = rt.tile([G4E, TPC], F32, tag="varb")
            nc.gpsimd.partition_all_reduce(varb, sq, channels=G4E, reduce_op=RED.add)
            rstd = rt.tile([G4E, TPC], F32, tag="rstd")
            nc.scalar.activation(
                rstd, varb, ACT.Sqrt, bias=eps_sb[:, :], scale=1.0 / G4E
            )
            nc.vector.reciprocal(rstd, rstd)
            nc.vector.tensor_tensor(out=d_sb, in0=d_sb, in1=rstd, op=ALU.mult)
            nc.vector.tensor_scalar(
                out=d_sb, in0=d_sb, scalar1=gamc_sb, scalar2=None, op0=ALU.mult
            )
            nc.vector.tensor_scalar(
                out=d_sb, in0=d_sb, scalar1=betc_sb, scalar2=None, op0=ALU.add
            )
            hn_bf = big.tile([G4E, TPC], BF16)
            nc.vector.tensor_scalar_max(hn_bf, d_sb, 0.0)

            gates = rt.tile([E, TPC], F32, tag="gates")
            for th in range(TH):
                sl = slice(th * 512, (th + 1) * 512)
                g_ps = psc.tile([E, 512], F32, tag="sm", name=f"g{th}")
                nc.tensor.matmul(g_ps, g2_sb, hn_bf[:, sl], start=True, stop=True)
                nc.vector.tensor_scalar(
                    out=gates[:, sl], in0=g_ps, scalar1=gb2c_sb,
                    scalar2=None, op0=ALU.add,
                )

            # ---- top-2 routing, expert-major ------------------------------
            v1 = rt.tile([E, TPC], F32, tag="v1")
            nc.gpsimd.partition_all_reduce(v1, gates, channels=E, reduce_op=RED.max)
            oh1 = rt.tile([E, TPC], F32, tag="oh1")
            nc.vector.tensor_tensor(out=oh1, in0=gates, in1=v1, op=ALU.is_ge)
            msk = rt.tile([E, TPC], F32, tag="msk")
            nc.vector.scalar_tensor_tensor(
                out=msk, in0=oh1, scalar=NEG, in1=gates, op0=ALU.mult, op1=ALU.add
            )
            v2 = rt.tile([E, TPC], F32, tag="v2")
            nc.gpsimd.partition_all_reduce(v2, msk, channels=E, reduce_op=RED.max)
            oh2 = rt.tile([E, TPC], F32, tag="oh2")
            nc.vector.tensor_tensor(out=oh2, in0=msk, in1=v2, op=ALU.is_ge)
            nc.vector.tensor_tensor(out=msk, in0=v1, in1=v2, op=ALU.subtract)
            s1 = rt.tile([E, TPC], F32, tag="s1")
            nc.scalar.activation(s1, msk, ACT.Sigmoid)
            u1 = rt.tile([E, TPC], F32, tag="u1")
            nc.vector.tensor_tensor(out=u1, in0=oh1, in1=s1, op=ALU.mult)
            u2 = rt.tile([E, TPC], F32, tag="u2")
            # u2 = oh2 * (1 - s1)
            nc.vector.scalar_tensor_tensor(
                out=u2, in0=s1, scalar=-1.0, in1=oh2, op0=ALU.mult, op1=ALU.add
            )
            nc.vector.tensor_tensor(out=u2, in0=u2, in1=oh2, op=ALU.mult)
            cnt = rt.tile([E, 2], F32, tag="cnt")
            nc.vector.tensor_reduce(out=cnt[:, 0:1], in_=oh1, axis=AX.X, op=ALU.add)
            nc.vector.tensor_reduce(out=cnt[:, 1:2], in_=oh2, axis=AX.X, op=ALU.add)
            cc_in = dp.tile([E, 2], F32)
            cc_out = dp.tile([E, 2], F32)
            nc.gpsimd.dma_start(cc_in, cnt)
            nc.gpsimd.collective_compute(
                "AllReduce",
                ALU.add,
                replica_groups=[list(range(N_CORES))],
                ins=[cc_in.opt()],
                outs=[cc_out.opt()],
            )
            cntg = rt.tile([E, 2], F32, tag="cntg")
            nc.gpsimd.dma_start(cntg, cc_out)

            # ---- unfused main blocks while the collective runs ------------
            for oc in range(4, FUSE_OC):
                main_oc(oc, False)

            # ---- post-collective combine (vector queue tail) --------------
            alw = rt.tile([E, 2], F32, tag="alw")
            nc.vector.tensor_scalar(
                out=alw, in0=cntg, scalar1=CAPACITY + 0.5, scalar2=None,
                op0=ALU.is_le,
            )
            q2 = rt.tile([E, TPC], F32, tag="q2")
            nc.vector.tensor_scalar(
                out=q2, in0=u2, scalar1=alw[:, 1:2], scalar2=None, op0=ALU.mult
            )
            w_bf = big.tile([E, TPC], BF16)
            nc.vector.scalar_tensor_tensor(
                out=w_bf, in0=u1, scalar=alw[:, 0:1], in1=q2,
                op0=ALU.mult, op1=ALU.add,
            )
            for th in range(TH):
                sl = slice(th * 512, (th + 1) * 512)
                wbr = psc.tile([128, 512], F32, tag="sm", name=f"wbr{th}")
                nc.tensor.matmul(wbr, repp_sb, w_bf[:, sl], start=True, stop=True)
                nc.vector.tensor_tensor(
                    out=tw_bf[:, sl], in0=tmp_sb[:, sl], in1=wbr, op=ALU.mult
                )

            # ---- fused main blocks ----------------------------------------
            for oc in range(FUSE_OC, OC):
                main_oc(oc, True)

            # ---- LoRA tail for the unfused blocks -------------------------
            for oc in range(FUSE_OC):
                for th in range(TH):
                    sl = slice(th * 512, (th + 1) * 512)
                    lp = psm.tile([128, 512], F32, tag="po", name=f"lp{oc}_{th}")
                    nc.tensor.matmul(
                        lp, b_sb[:, oc * 128 : (oc + 1) * 128], tw_bf[:, sl],
                        start=True, stop=True,
                    )
                    lsb = op_.tile([128, 512], F32, tag="lsb")
                    if th == 0:
                        nc.scalar.activation(lsb, lp, ACT.Copy)
                    else:
                        nc.vector.tensor_copy(lsb, lp)
                    nc.sync.dma_start(
                        loraT.ap()[oc * 128 : (oc + 1) * 128, sl], lsb
                    )
    return nc


_CACHE = {}


def _get_nc():
    if "nc" not in _CACHE:
        nc = build_bass()
        nc.finalize()
        _CACHE["nc"] = nc
    return _CACHE["nc"]


def prep_in_maps(inputs):
    x = np.asarray(inputs["x"], dtype=np.float32)
    weight = np.asarray(inputs["weight"], dtype=np.float32)
    xf = x.reshape(N_TOK, IN)
    # wp[oc, p, k*128+c] = weight[oc*128+c, k*128+p]
    wp = np.ascontiguousarray(
        weight.reshape(OC, 128, KT, 128).transpose(0, 3, 2, 1).reshape(OC, 128, KT * 128)
    ).astype(BF)
    a_cat = (
        np.asarray(inputs["lora_A"], np.float32).transpose(1, 0, 2).reshape(IN, ER)
        * ALPHA
    )
    ap_ = np.ascontiguousarray(
        a_cat.reshape(KT, 128, ER).transpose(1, 0, 2).reshape(128, KT * ER)
    ).astype(BF)
    # centered gate weights: LN mean subtraction folded into G1 and gb1
    g1T = np.asarray(inputs["gw1"], np.float32).T  # [IN, 32]
    g1T = g1T - g1T.mean(axis=1, keepdims=True)
    g1p = np.ascontiguousarray(
        g1T.reshape(KT, 128, G4E).transpose(1, 0, 2).reshape(128, KT * G4E)
    ).astype(BF)
    gb1 = np.asarray(inputs["gb1"], np.float32)
    gb1 = gb1 - gb1.mean()
    bp = np.asarray(inputs["lora_B"], np.float32).reshape(ER, OUT).astype(BF)
    g2p = np.ascontiguousarray(np.asarray(inputs["gw2"], np.float32).T).astype(BF)
    repm = np.zeros((E, ER), np.float32)
    for e in range(E):
        repm[e, e * R : (e + 1) * R] = 1.0
    repp = repm.astype(BF)
    gb1c = np.ascontiguousarray(gb1.reshape(G4E, 1))
    gamc = np.ascontiguousarray(
        np.asarray(inputs["ln_gamma"], np.float32).reshape(G4E, 1)
    )
    betc = np.ascontiguousarray(
        np.asarray(inputs["ln_beta"], np.float32).reshape(G4E, 1)
    )
    gb2c = np.ascontiguousarray(np.asarray(inputs["gb2"], np.float32).reshape(E, 1))

    shared = dict(
        wp=wp, ap_=ap_, g1p=g1p, bp=bp, g2p=g2p, repp=repp,
        gb1c=gb1c, gamc=gamc, betc=betc, gb2c=gb2c,
    )
    in_maps = []
    for c in range(N_CORES):
        xs = xf[c * TPC : (c + 1) * TPC]  # [TPC, IN]
        xpc = np.ascontiguousarray(
            xs.T.reshape(KT, 128, TPC).transpose(1, 0, 2).reshape(128, KT * TPC)
        ).astype(BF)
        in_maps.append(dict(xp=xpc, **shared))
    return in_maps


def gather(results):
    out = np.empty((N_TOK, OUT), np.float32)
    for c in range(N_CORES):
        tot = np.array(results[c]["outT"])
        tot[: FUSE_OC * 128] += results[c]["loraT"]
        out[c * TPC : (c + 1) * TPC] = tot.T
    return out.reshape(B, S, OUT)


def kernel(**inputs):
    in_maps = prep_in_maps(inputs)
    nc = _get_nc()
    res = run_bass_kernel_spmd(nc, in_maps, core_ids=list(range(N_CORES)))
    return gather(res.results)
